# revision 45
# baseline (speedup 1.0000x reference)
"""Trainium2 Bass kernel for nn_DomainAdaption (conv-conv-MoE-gated-residual).

Data-parallel over batch: 16 samples -> 8 NeuronCores, 2 samples/core.

Everything heavy runs through fp8e4 DoubleRow matmuls (0.5 cyc/row, 2x128
contraction per instruction).  Images are stored with row stride 128 (NO
column padding) so each 4-row conv window is one contiguous 512-element
run -- the DR moving operand is then a clean [128, 2, 512] AP whose pair
dim selects two conv taps (pair strides must be 0, 2 or >=128: stride 1
wedges the PE, hence pairs (t0,t3)(t1,t4)(t2,t5)(t6,t8)(t7,-)).
Horizontal padding is emulated: the wrap-around garbage that taps dx=0 /
dx=2 read at columns 0/127 is recomputed into a tiny contiguous psum tile
by 4 fix-up matmuls per group, staged to SBUF (DVE may read only one PSUM
operand), and subtracted from the psum edge columns by one DVE op.
Vertical padding is real (zero rows), plus guard rows front/back.

Per sample:
  conv1: 5 DR pair-matmuls per [4x128] chunk over fp8(16*x); ScalarE Prelu
         epilogue (scale 1/256) writes h1 fp8 + pooling partials
         (accum_out).
  gate EARLY (before conv2): mean(conv2(h1)) is computed exactly from 9
         reduced h1 vectors (sum S, edge strips, corners) via
         inclusion-exclusion over the conv window, then 9 tiny f32
         stat-matmuls + adapter MLP + a broadcast matmul + sigmoid
         (all on-device; hides under the other sample's conv phase).
  conv2: the gate is folded into the fp8 conv2 weights on-device
         (w2g = fp8(16*w2*g[o]), one GPSIMD multiply against a
         PE-broadcast gate plane) and the residual x is injected INTO the
         conv2 PSUM as a DR pair (I @ fp8(16x) + I @ fp8 correction), so
         a single Prelu epilogue (scale 1/16) emits
         y = prelu(g*conv2(h1) + x) in bf16 directly -- h2 is never
         materialized and there is no separate residual pass.

PE order c1(s0), c1(s1), c2(s0), c2(s1) with stats/gate/fold of each
sample emitted mid-phase of the other sample, so the PE never waits on
the gate chain.  285us (baseline) -> 101us measured on TimelineSim;
hardware rel err 9.9e-3 vs the fp32 reference.
"""
import sys

if "/opt/trn_rl_repo" not in sys.path:
    sys.path.insert(0, "/opt/trn_rl_repo")

import numpy as np
import ml_dtypes

N, C, H, W = 16, 128, 128, 128
CH = 32
NCORES = 8
SPC = N // NCORES          # samples per core
RS = W                     # stored row stride
SR = H + 4                 # stored rows: guard, zero, 128 data, zero, guard
GP = SR * RS               # elements per stored plane (16896)
SC = 16.0                  # fp8 scale for x and conv weights
BF = ml_dtypes.bfloat16
E4NP = ml_dtypes.float8_e4m3fn

# DoubleRow tap pairs: (tap_a, tap_b); taps are t = 3*dy + dx.
PAIRS = [(0, 3), (1, 4), (2, 5), (6, 8), (7, None)]


def _build(prelu1: float, prelu2: float):
    import concourse.mybir as mybir
    import concourse.tile as tile
    from concourse import bacc
    import bass_rust

    F32 = mybir.dt.float32
    F32R = mybir.dt.float32r
    BF16 = mybir.dt.bfloat16
    E4 = mybir.dt.float8e4
    AF = mybir.ActivationFunctionType
    ALU = mybir.AluOpType
    PM = mybir.MatmulPerfMode
    V = bass_rust.VecI64Pair

    nc = bacc.Bacc("TRN2", target_bir_lowering=False, debug=False,
                   num_devices=NCORES)

    xc_d = nc.dram_tensor("xc", [SPC, C, 2, GP], E4, kind="ExternalInput").ap()
    cw1_d = nc.dram_tensor("cw1", [C, 5, 2, C], E4, kind="ExternalInput").ap()
    w2m_d = nc.dram_tensor("w2m", [C, 5, 2, C], BF16, kind="ExternalInput").ap()
    w2s_d = nc.dram_tensor("w2s", [C, 9, C], F32, kind="ExternalInput").ap()
    i2_d = nc.dram_tensor("i2", [C, 2, C], E4, kind="ExternalInput").ap()
    ones1_d = nc.dram_tensor("ones1", [1, C], F32, kind="ExternalInput").ap()
    c1b_d = nc.dram_tensor("c1b", [C, 1], F32, kind="ExternalInput").ap()
    w1pT_d = nc.dram_tensor("w1pT", [SPC, C, CH], F32, kind="ExternalInput").ap()
    b1g_d = nc.dram_tensor("b1g", [SPC, CH, 1], F32, kind="ExternalInput").ap()
    w2aT_d = nc.dram_tensor("w2aT", [SPC, CH + 1, C], F32, kind="ExternalInput").ap()
    y_d = nc.dram_tensor("y", [SPC, C, H, W], BF16, kind="ExternalOutput").ap()

    with tile.TileContext(nc) as tc, (
        tc.tile_pool(name="wp", bufs=1)) as wp, (
        tc.tile_pool(name="xp", bufs=1)) as xp, (
        tc.tile_pool(name="hp", bufs=1)) as hpool, (
        tc.tile_pool(name="sp", bufs=1)) as spool, (
        tc.tile_pool(name="yp", bufs=4)) as ypool, (
        tc.tile_pool(name="pc", bufs=3, space="PSUM")) as pc, (
        tc.tile_pool(name="pv", bufs=1, space="PSUM")) as pv, (
        tc.tile_pool(name="pf", bufs=1, space="PSUM")) as pf:

        # --- static weights / constants ---
        cw1_t = wp.tile([C, 5, 2, C], E4, name="cw1t")
        w2m_t = wp.tile([C, 5, 2, C], BF16, name="w2mt")
        w2s_t = wp.tile([C, 9, C], F32, name="w2st")
        i2_t = wp.tile([C, 2, C], E4, name="i2t")
        ones1_t = wp.tile([1, C], F32, name="ones1t")
        c1b_t = wp.tile([C, 1], F32, name="c1bt")
        z9_t = wp.tile([C, 9], F32, name="z9t")
        strash = wp.tile([C, 32 * RS], E4, name="strash")
        nc.vector.memset(z9_t[:], 0)

        # per-sample tiles
        xc = [xp.tile([C, 2, GP], E4, name=f"xc{s}") for s in range(SPC)]
        h1 = [hpool.tile([C, GP], E4, name=f"h1_{s}") for s in range(SPC)]
        w2g = [wp.tile([C, 5, 2, C], E4, name=f"w2g{s}") for s in range(SPC)]
        w1pT_t = [wp.tile([C, CH], F32, name=f"w1pT{s}") for s in range(SPC)]
        b1g_t = [wp.tile([CH, 1], F32, name=f"b1g{s}") for s in range(SPC)]
        w2aT_t = [wp.tile([CH + 1, C], F32, name=f"w2aT{s}") for s in range(SPC)]
        spart = [spool.tile([C, 16], F32, name=f"spart{s}") for s in range(SPC)]
        sig = [spool.tile([C, 9], F32, name=f"sig{s}") for s in range(SPC)]
        red = [spool.tile([C, 4], F32, name=f"red{s}") for s in range(SPC)]
        x1sb = [spool.tile([C, 1], F32, name=f"x1sb{s}") for s in range(SPC)]
        a_aug = [spool.tile([CH + 1, 1], F32, name=f"aaug{s}") for s in range(SPC)]
        gprer = [spool.tile([1, C], F32, name=f"gprer{s}") for s in range(SPC)]
        gb = [spool.tile([C, C], F32, name=f"gb{s}") for s in range(SPC)]

        for s in range(SPC):
            # guard + zero rows of h1 (interior rewritten every sample)
            nc.vector.memset(h1[s][:, 0:2 * RS], 0)
            nc.vector.memset(h1[s][:, (SR - 2) * RS:], 0)
            nc.vector.memset(a_aug[s][CH:CH + 1, :], 1.0)

        # --- DMAs (SP queue) ---
        # plane-0 (conv input) bands first so PE starts asap; the C16
        # correction plane is only needed by the conv2 inject, so it
        # streams later.  Band k covers stored rows 33k .. 33k+36.
        def x_bands(s, ks, pl):
            for k in ks:
                a = RS * 33 * k
                b = min(GP, RS * (33 * k + 37))
                m = (a + b) // (2 * RS) * RS
                nc.sync.dma_start(xc[s][:, pl, a:m], xc_d[s, :, pl, a:m])
                nc.sync.dma_start(xc[s][:, pl, m:b], xc_d[s, :, pl, m:b])

        nc.sync.dma_start(xc[0][:, 0, 0:RS * 11], xc_d[0, :, 0, 0:RS * 11])
        nc.sync.dma_start(cw1_t[:], cw1_d)
        nc.sync.dma_start(xc[0][:, 0, RS * 11:RS * 23],
                          xc_d[0, :, 0, RS * 11:RS * 23])
        nc.sync.dma_start(xc[0][:, 0, RS * 23:RS * 37],
                          xc_d[0, :, 0, RS * 23:RS * 37])
        for k in range(1, 4):
            a, b = RS * 33 * k, min(GP, RS * (33 * k + 37))
            nc.scalar.dma_start(xc[0][:, 0, a:b], xc_d[0, :, 0, a:b])
        nc.sync.dma_start(c1b_t[:], c1b_d)
        x_bands(1, range(4), 0)
        nc.sync.dma_start(i2_t[:], i2_d)
        nc.sync.dma_start(w2m_t[:], w2m_d)
        nc.sync.dma_start(w2s_t[:], w2s_d)
        nc.sync.dma_start(ones1_t[:], ones1_d)
        for s in range(SPC):
            nc.sync.dma_start(w1pT_t[s][:], w1pT_d[s])
            nc.sync.dma_start(b1g_t[s][:], b1g_d[s])
            nc.sync.dma_start(w2aT_t[s][:], w2aT_d[s])
        for s in range(SPC):
            for (a, b) in ((0, GP // 2), (GP // 2, GP)):
                nc.sync.dma_start(xc[s][:, 1, a:b], xc_d[s, :, 1, a:b])

        def conv_group(s, g, base_ap, pstride, wt, inj_base=None,
                       split=False):
            """10 DR tap matmuls (+2 injects) + wrap fix-ups -> 2-bank psum.

            base_ap: AP anchored at the image plane start.  With split=True
            each psum bank gets its own edge fix so the epilogue can drain
            bank 0 while the PE still fills bank 1 (shorter tail)."""
            pp = pc.tile([C, 2, 4, W], F32, name="pp")
            b0 = base_ap.offset
            ff = pf.tile([C, 2, 2, 4], F32, name="ff")
            pstr = pp[:].ap[0][0]

            def half(h):
                c = 2 * g + h
                for p in range(5):
                    ta, tb = PAIRS[p]
                    dy0, dx0 = ta // 3, ta % 3
                    dstr = 0 if tb is None else (
                        (tb // 3 - dy0) * RS + (tb % 3 - dx0))
                    rhs = base_ap.copy()
                    rhs.ap = V([[pstride, C], [dstr, 2], [1, 4 * W]])
                    rhs.offset = b0 + (4 * c + dy0 + 1) * RS + dx0 - 1
                    nc.tensor.matmul(pp[:, h], wt[:, p], rhs,
                                     start=(p == 0), stop=False,
                                     perf_mode=PM.DoubleRow,
                                     skip_group_check=True)
                if inj_base is not None:   # conv2: inject residual x
                    inj = inj_base.copy()
                    inj.ap = V([[2 * GP, C], [GP, 2], [1, 4 * W]])
                    inj.offset = inj_base.offset + (4 * c + 2) * RS
                    nc.tensor.matmul(pp[:, h], i2_t[:], inj,
                                     start=False, stop=False,
                                     perf_mode=PM.DoubleRow,
                                     skip_group_check=True)

            def fix(h):
                # wrap-around garbage at out cols 0/127: accumulate garbage
                # into contiguous psum F, then subtract from the edge cols.
                # col 0: taps (dy,0) = pair0 + single t6 (= wt[:,3,0]);
                # col 127: taps (dy,2) = pair2 + single t8 (= wt[:,3,1]).
                for side, (pair_p, single_sl, coff) in enumerate(
                        ((0, (3, 0), -1), (2, (3, 1), RS))):
                    fo = ff[:, h, side]
                    src = base_ap.copy()
                    src.ap = V([[pstride, C], [RS, 2], [RS, 4]])
                    src.offset = b0 + (8 * g + 4 * h + 1) * RS + coff
                    nc.tensor.matmul(fo, wt[:, pair_p], src,
                                     start=True, stop=False,
                                     perf_mode=PM.DoubleRow,
                                     skip_group_check=True)
                    src2 = base_ap.copy()
                    src2.ap = V([[pstride, C], [RS, 4]])
                    src2.offset = b0 + (8 * g + 4 * h + 3) * RS + coff
                    nc.tensor.matmul(fo, wt[:, single_sl[0], single_sl[1]],
                                     src2, start=False, stop=True,
                                     skip_group_check=True)

            def merge(hs):
                # DVE may read only ONE operand from PSUM: stage F in SBUF
                nh = len(hs)
                fs = spool.tile([C, 16], F32, name="fs", bufs=4)
                nc.vector.tensor_scalar(
                    fs[:, 8 * hs[0]:8 * hs[0] + 8 * nh],
                    ff[:, hs[0]:hs[0] + nh].rearrange(
                        "p a b c -> p (a b c)"), 0.0, None, ALU.add)
                edge = pp[:].copy()
                edge.ap = V([[pstr, C], [512, nh], [128, 4], [W - 1, 2]])
                edge.offset = pp[:].offset + 512 * hs[0]
                fap = fs[:, 0:1].copy()
                fap.ap = V([[16, C], [8, nh], [1, 4], [4, 2]])
                fap.offset = fap.offset + 8 * hs[0]
                nc.vector.tensor_tensor(edge, edge, fap, op=ALU.subtract)

            if split:
                half(0); fix(0); merge([0])
                half(1); fix(1); merge([1])
            else:
                half(0); half(1); fix(0); fix(1); merge([0, 1])
            return pp

        def conv1_group(s, g):
            pp = conv_group(s, g, xc[s][:, 0, 0:1], 2 * GP, cw1_t)
            a0 = (8 * g + 2) * RS
            out = h1[s][:, a0:a0 + 8 * RS].rearrange(
                "p (a b w) -> p a b w", a=2, b=4)
            if False and 0.0 <= prelu1 <= 1.0:
                # DVE epilogue relieves the Act queue at conv1 phase ends
                ct = ypool.tile([C, 2, 4, W], BF16, name="c1t")
                nc.vector.tensor_scalar(ct[:], pp[:], 1.0 / (SC * SC),
                                        c1b_t[:], ALU.mult, ALU.add)
                nc.vector.scalar_tensor_tensor(out, ct[:], prelu1, ct[:],
                                               op0=ALU.mult, op1=ALU.max)
                nc.vector.tensor_reduce(spart[s][:, g:g + 1],
                                        h1[s][:, a0:a0 + 8 * RS],
                                        axis=mybir.AxisListType.X,
                                        op=ALU.add)
            else:
                nc.scalar.activation(out, pp[:], AF.Prelu, bias=c1b_t[:],
                                     scale=1.0 / (SC * SC), alpha=prelu1,
                                     accum_out=spart[s][:, g:g + 1])

        def conv2_group(s, g, on_dve, split=False, tail=False):
            pp = conv_group(s, g, h1[s][:, 0:1], GP, w2g[s],
                            inj_base=xc[s][:, 0, 0:1], split=split)
            if tail:
                # split=True ordered the halves' fixes/merges separately:
                # drain each bank through Act + sync store as soon as ready
                for h in range(2):
                    yh = ypool.tile([C, 4, W], BF16, name="yh")
                    nc.scalar.activation(yh[:], pp[:, h], AF.Prelu,
                                         scale=1.0 / SC, alpha=prelu2)
                    nc.sync.dma_start(y_d[s, :, 8 * g + 4 * h:
                                          8 * g + 4 * h + 4, :], yh[:])
                return
            if split:
                # per-half epilogue + store on disjoint engines/queues:
                # drains bank 0 while the PE still fills bank 1
                yh = ypool.tile([C, 4, W], BF16, name="yh")
                nc.scalar.activation(yh[:], pp[:, 0], AF.Prelu,
                                     scale=1.0 / SC, alpha=prelu2)
                nc.sync.dma_start(y_d[s, :, 8 * g:8 * g + 4, :], yh[:])
                th = ypool.tile([C, 4, W], BF16, name="th")
                nc.vector.tensor_scalar(th[:], pp[:, 1], 1.0 / SC, None,
                                        ALU.mult)
                yh2 = ypool.tile([C, 4, W], BF16, name="yh2")
                nc.vector.scalar_tensor_tensor(yh2[:], th[:], prelu2, th[:],
                                               op0=ALU.mult, op1=ALU.max)
                nc.gpsimd.dma_start(y_d[s, :, 8 * g + 4:8 * g + 8, :],
                                    yh2[:])
                return
            yt = ypool.tile([C, 2, 4, W], BF16, name="yt")
            if on_dve:
                tt = ypool.tile([C, 2, 4, W], BF16, name="tt")
                nc.vector.tensor_scalar(tt[:], pp[:], 1.0 / SC, None, ALU.mult)
                nc.vector.scalar_tensor_tensor(yt[:], tt[:], prelu2, tt[:],
                                               op0=ALU.mult, op1=ALU.max)
            else:
                nc.scalar.activation(yt[:], pp[:], AF.Prelu,
                                     scale=1.0 / SC, alpha=prelu2)
            nc.sync.dma_start(
                y_d[s, :, 8 * g:8 * g + 8, :].rearrange(
                    "p (a b) w -> p a b w", a=2), yt[:])

        def pool_ssum(s, q):
            # pooling sum partial over data rows 32q..32q+31 on idle GPSIMD:
            # copy-to-trash with accum_out gives the free-axis sum
            a = (2 + 32 * q) * RS
            nc.gpsimd.tensor_scalar(strash[:], h1[s][:, a:a + 32 * RS],
                                    1.0, None, ALU.mult,
                                    accum_out=spart[s][:, q:q + 1])

        def stats_sigma(s):
            """strips + corners + sigma build (DVE), inclusion-exclusion."""
            hs = h1[s]
            X = mybir.AxisListType.X
            nc.vector.tensor_reduce(red[s][:, 0:1], spart[s][:], axis=X,
                                    op=ALU.add)                           # S (from Pool partials)
            nc.vector.tensor_reduce(red[s][:, 1:2], hs[:, 2 * RS:3 * RS],
                                    axis=X, op=ALU.add)                   # Rt
            nc.vector.tensor_reduce(red[s][:, 2:3],
                                    hs[:, (SR - 3) * RS:(SR - 2) * RS],
                                    axis=X, op=ALU.add)                   # Rb
            cl = hs[:, 0:1].copy()
            cl.ap = V([[GP, C], [RS, H]])
            cl.offset = cl.offset + 2 * RS
            nc.vector.tensor_reduce(red[s][:, 3:4], cl, axis=X, op=ALU.add)  # Cl
            crt = spool.tile([C, 1], F32, name=f"cr{s}")
            cr = hs[:, 0:1].copy()
            cr.ap = V([[GP, C], [RS, H]])
            cr.offset = cr.offset + 2 * RS + (W - 1)
            nc.vector.tensor_reduce(crt[:], cr, axis=X, op=ALU.add)          # Cr
            sg = sig[s]
            nc.vector.tensor_scalar(sg[:], z9_t[:], red[s][:, 0:1], None,
                                    ALU.add)
            nc.vector.tensor_scalar(sg[:, 0:3], sg[:, 0:3], red[s][:, 2:3],
                                    None, ALU.subtract)                   # dy=0: -Rb
            nc.vector.tensor_scalar(sg[:, 6:9], sg[:, 6:9], red[s][:, 1:2],
                                    None, ALU.subtract)                   # dy=2: -Rt
            for col0, which in ((0, crt[:]), (2, red[s][:, 3:4])):
                ap = sg[:, col0:col0 + 1].copy()
                ap.ap = V([[9, C], [3, 3]])
                nc.vector.tensor_scalar(ap, ap, which, None, ALU.subtract)
            # corners [(0,0),(0,127),(127,0),(127,127)]: strided fp8->f32 copy
            corn = spool.tile([C, 2, 2], F32, name=f"corn{s}")
            cap = hs[:, 0:1].copy()
            cap.ap = V([[GP, C], [(H - 1) * RS, 2], [W - 1, 2]])
            cap.offset = cap.offset + 2 * RS
            nc.vector.tensor_scalar(corn[:], cap, 0.0, None, ALU.add)
            cf = corn[:].rearrange("p a b -> p (a b)")
            for t, ci in ((8, 0), (6, 1), (2, 2), (0, 3)):
                nc.vector.tensor_scalar(sg[:, t:t + 1], sg[:, t:t + 1],
                                        cf[:, ci:ci + 1], None, ALU.add)

        def gate_mlp(s):
            t1 = pv.tile([C, C], F32, name="pvs")
            psx, psa = t1[:, 0:1], t1[0:CH, 1:2]
            for t in range(9):
                nc.tensor.matmul(psx, w2s_t[:, t], sig[s][:, t:t + 1],
                                 start=(t == 0), stop=(t == 8))
            nc.scalar.activation(x1sb[s][:], psx, AF.Copy)
            nc.tensor.matmul(psa, w1pT_t[s][:], x1sb[s][:],
                             start=True, stop=True)
            nc.scalar.activation(a_aug[s][0:CH, :], psa, AF.Relu,
                                 bias=b1g_t[s][:])
            psg = pv.tile([C, C], F32, name="pvs")[0:1, :]
            nc.tensor.matmul(psg, a_aug[s][:], w2aT_t[s][:],
                             start=True, stop=True)
            nc.scalar.activation(gprer[s][:], psg, AF.Copy)
            psb = pv.tile([C, C], F32, name="pvs")
            nc.tensor.matmul(psb[:], ones1_t[:], gprer[s][:],
                             start=True, stop=True)
            nc.scalar.activation(gb[s][:], psb[:], AF.Sigmoid)

        def fold_w2(s):
            def bc(shape_dims):
                ap = gb[s][:, 0:1].copy()
                ap.ap = V([[C, C]] + shape_dims)
                return ap
            nc.gpsimd.tensor_tensor(w2g[s][:], w2m_t[:],
                                    bc([[0, 5], [0, 2], [1, C]]), op=ALU.mult)

        # ---------------- emission schedule ----------------
        for g in range(16):
            conv1_group(0, g)
        stats_sigma(0)
        for g in range(8):
            conv1_group(1, g)
        gate_mlp(0)
        for g in range(8, 12):
            conv1_group(1, g)
        fold_w2(0)
        for g in range(12, 16):
            conv1_group(1, g)
        stats_sigma(1)
        for g in range(8):
            conv2_group(0, g, on_dve=False)
        gate_mlp(1)
        for g in range(8, 12):
            conv2_group(0, g, on_dve=False)
        fold_w2(1)
        for g in range(12, 16):
            conv2_group(0, g, on_dve=False)
        for g in range(15):
            conv2_group(1, g, on_dve=False)
        conv2_group(1, 15, on_dve=False, split=True, tail=True)

    nc.compile()
    return nc


_CACHE = {}


def _get_program(prelu1, prelu2):
    key = (float(prelu1), float(prelu2))
    if key not in _CACHE:
        _CACHE[key] = _build(*key)
    return _CACHE[key]


def _prep(x, intensity, conv1_w, conv1_b, prelu1, conv2_w, conv2_b,
          aW1, ab1, aW2, ab2, prelu2):
    x = np.asarray(x, np.float32)
    idx = np.asarray(intensity).astype(np.int64) - 1
    conv1_w = np.asarray(conv1_w, np.float32)
    conv1_b = np.asarray(conv1_b, np.float32)
    conv2_w = np.asarray(conv2_w, np.float32)
    conv2_b = np.asarray(conv2_b, np.float32)
    aW1 = np.asarray(aW1, np.float32)
    ab1 = np.asarray(ab1, np.float32)
    aW2 = np.asarray(aW2, np.float32)
    ab2 = np.asarray(ab2, np.float32)
    assert not np.any(conv2_b), "conv2 bias folding not implemented"

    # stored planes: guard row, zero row, 128 data rows, zero row, guard row
    xpad = np.zeros((N, C, SR, RS), np.float32)
    xpad[:, :, 2:H + 2, :] = x * SC
    x16 = xpad.astype(E4NP)
    c16 = (xpad - x16.astype(np.float32)).astype(E4NP)
    xc = np.stack([x16, c16], axis=2).reshape(N, C, 2, GP)

    wtap1 = conv1_w.transpose(1, 2, 3, 0).reshape(C, 9, C)  # [i, t, o]
    wtap2 = conv2_w.transpose(1, 2, 3, 0).reshape(C, 9, C)

    def pair_pack(wtap, dtype):
        out = np.zeros((C, 5, 2, C), np.float32)
        for p, (ta, tb) in enumerate(PAIRS):
            out[:, p, 0] = wtap[:, ta]
            if tb is not None:
                out[:, p, 1] = wtap[:, tb]
        return np.ascontiguousarray((out * SC).astype(dtype))

    cw1 = pair_pack(wtap1, E4NP)
    w2m = pair_pack(wtap2, BF)
    w2s = np.ascontiguousarray(wtap2)
    i2 = np.zeros((C, 2, C), np.float32)
    i2[:, 0] = np.eye(C)
    i2[:, 1] = np.eye(C)
    i2 = i2.astype(E4NP)

    w1pT = np.ascontiguousarray(
        (aW1[idx] / (SC * H * W)).transpose(0, 2, 1))     # [N, C, CH]
    b1g = np.ascontiguousarray(ab1[idx])[:, :, None]      # [N, CH, 1]
    w2aT = np.concatenate(
        [aW2[idx].transpose(0, 2, 1), ab2[idx][:, None, :]], axis=1)

    nc = _get_program(float(prelu1), float(prelu2))

    in_maps = []
    for i in range(NCORES):
        sl = slice(i * SPC, (i + 1) * SPC)
        in_maps.append(dict(
            xc=xc[sl], cw1=cw1, w2m=w2m, w2s=w2s, i2=i2,
            ones1=np.ones((1, C), np.float32),
            c1b=conv1_b[:, None],
            w1pT=np.ascontiguousarray(w1pT[sl]),
            b1g=np.ascontiguousarray(b1g[sl]),
            w2aT=np.ascontiguousarray(w2aT[sl])))
    return nc, in_maps


def kernel(**inputs):
    import time
    from concourse.bass_utils import run_bass_kernel_spmd

    nc, in_maps = _prep(**inputs)
    res = None
    for attempt, pause in enumerate((0, 15, 60, 120)):
        if pause:
            time.sleep(pause)
        try:
            res = run_bass_kernel_spmd(nc, in_maps,
                                       core_ids=list(range(NCORES)))
            break
        except Exception:
            # transient NRT_EXEC_UNIT_UNRECOVERABLE (wedged core); retry
            if attempt == 3:
                raise
    return np.concatenate(
        [r["y"].astype(np.float32) for r in res.results], axis=0)


# revision 52
# speedup vs baseline: 1.0261x; 1.0261x over previous
"""Trainium2 Bass kernel for nn_DomainAdaption (conv-conv-MoE-gated-residual).

Data-parallel over batch: 16 samples -> 8 NeuronCores, 2 samples/core.

Everything heavy runs through fp8e4 DoubleRow matmuls (0.5 cyc/row, 2x128
contraction per instruction).  Images are stored with row stride 128 (NO
column padding) so each 4-row conv window is one contiguous 512-element
run -- the DR moving operand is then a clean [128, 2, 512] AP whose pair
dim selects two conv taps (pair strides must be 0, 2 or >=128: stride 1
wedges the PE, hence pairs (t0,t3)(t1,t4)(t2,t5)(t6,t8)(t7,-)).
Horizontal padding is emulated: the wrap-around garbage that taps dx=0 /
dx=2 read at columns 0/127 is recomputed into a tiny contiguous psum tile
by 4 fix-up matmuls per group, staged to SBUF (DVE may read only one PSUM
operand), and subtracted from the psum edge columns by one DVE op.
Vertical padding is real (zero rows), plus guard rows front/back.

Per sample:
  conv1: 5 DR pair-matmuls per [4x128] chunk over fp8(16*x); ScalarE Prelu
         epilogue (scale 1/256) writes h1 fp8 + pooling partials
         (accum_out).
  gate EARLY (before conv2): mean(conv2(h1)) is computed exactly from 9
         reduced h1 vectors (sum S, edge strips, corners) via
         inclusion-exclusion over the conv window, then 9 tiny f32
         stat-matmuls + adapter MLP + a broadcast matmul + sigmoid
         (all on-device; hides under the other sample's conv phase).
  conv2: the gate is folded into the fp8 conv2 weights on-device
         (w2g = fp8(16*w2*g[o]), one GPSIMD multiply against a
         PE-broadcast gate plane) and the residual x is injected INTO the
         conv2 PSUM as a DR pair (I @ fp8(16x) + I @ fp8 correction), so
         a single Prelu epilogue (scale 1/16) emits
         y = prelu(g*conv2(h1) + x) in bf16 directly -- h2 is never
         materialized and there is no separate residual pass.

PE order c1(s0), c1(s1), c2(s0), c2(s1) with stats/gate/fold of each
sample emitted mid-phase of the other sample, so the PE never waits on
the gate chain.  285us (baseline) -> 101us measured on TimelineSim;
hardware rel err 9.9e-3 vs the fp32 reference.
"""
import sys

if "/opt/trn_rl_repo" not in sys.path:
    sys.path.insert(0, "/opt/trn_rl_repo")

import numpy as np
import ml_dtypes

N, C, H, W = 16, 128, 128, 128
CH = 32
NCORES = 8
SPC = N // NCORES          # samples per core
RS = W                     # stored row stride
SR = H + 4                 # stored rows: guard, zero, 128 data, zero, guard
GP = SR * RS               # elements per stored plane (16896)
SC = 16.0                  # fp8 scale for x and conv weights
BF = ml_dtypes.bfloat16
E4NP = ml_dtypes.float8_e4m3fn

# DoubleRow tap pairs: (tap_a, tap_b); taps are t = 3*dy + dx.
PAIRS = [(0, 3), (1, 4), (2, 5), (6, 8), (7, None)]


def _build(prelu1: float, prelu2: float):
    import concourse.mybir as mybir
    import concourse.tile as tile
    from concourse import bacc
    import bass_rust

    F32 = mybir.dt.float32
    F32R = mybir.dt.float32r
    BF16 = mybir.dt.bfloat16
    E4 = mybir.dt.float8e4
    AF = mybir.ActivationFunctionType
    ALU = mybir.AluOpType
    PM = mybir.MatmulPerfMode
    V = bass_rust.VecI64Pair

    nc = bacc.Bacc("TRN2", target_bir_lowering=False, debug=False,
                   num_devices=NCORES)

    xc_d = nc.dram_tensor("xc", [SPC, C, 2, GP], E4, kind="ExternalInput").ap()
    cw1_d = nc.dram_tensor("cw1", [C, 5, 2, C], E4, kind="ExternalInput").ap()
    w2m_d = nc.dram_tensor("w2m", [C, 5, 2, C], BF16, kind="ExternalInput").ap()
    w2s_d = nc.dram_tensor("w2s", [C, 9, C], F32, kind="ExternalInput").ap()
    i2_d = nc.dram_tensor("i2", [C, 2, C], E4, kind="ExternalInput").ap()
    ones1_d = nc.dram_tensor("ones1", [1, C], F32, kind="ExternalInput").ap()
    c1b_d = nc.dram_tensor("c1b", [C, 1], F32, kind="ExternalInput").ap()
    w1pT_d = nc.dram_tensor("w1pT", [SPC, C, CH], F32, kind="ExternalInput").ap()
    b1g_d = nc.dram_tensor("b1g", [SPC, CH, 1], F32, kind="ExternalInput").ap()
    w2aT_d = nc.dram_tensor("w2aT", [SPC, CH + 1, C], F32, kind="ExternalInput").ap()
    y_d = nc.dram_tensor("y", [SPC, C, H, W], BF16, kind="ExternalOutput").ap()

    with tile.TileContext(nc) as tc, (
        tc.tile_pool(name="wp", bufs=1)) as wp, (
        tc.tile_pool(name="xp", bufs=1)) as xp, (
        tc.tile_pool(name="hp", bufs=1)) as hpool, (
        tc.tile_pool(name="sp", bufs=1)) as spool, (
        tc.tile_pool(name="yp", bufs=4)) as ypool, (
        tc.tile_pool(name="pc", bufs=3, space="PSUM")) as pc, (
        tc.tile_pool(name="pv", bufs=1, space="PSUM")) as pv, (
        tc.tile_pool(name="pf", bufs=1, space="PSUM")) as pf:

        # --- static weights / constants ---
        cw1_t = wp.tile([C, 5, 2, C], E4, name="cw1t")
        w2m_t = wp.tile([C, 5, 2, C], BF16, name="w2mt")
        w2s_t = wp.tile([C, 9, C], F32, name="w2st")
        i2_t = wp.tile([C, 2, C], E4, name="i2t")
        ones1_t = wp.tile([1, C], F32, name="ones1t")
        c1b_t = wp.tile([C, 1], F32, name="c1bt")
        z9_t = wp.tile([C, 9], F32, name="z9t")
        strash = wp.tile([C, 32 * RS], E4, name="strash")
        wdum = wp.tile([C, 2, C], E4, name="wdum")
        zdum = wp.tile([C, 2, C], E4, name="zdum")
        nc.vector.memset(wdum[:], 0)
        nc.vector.memset(zdum[:], 0)
        nc.vector.memset(z9_t[:], 0)

        # per-sample tiles
        xc = [xp.tile([C, 2, GP], E4, name=f"xc{s}") for s in range(SPC)]
        h1 = [hpool.tile([C, GP], E4, name=f"h1_{s}") for s in range(SPC)]
        w2g = [wp.tile([C, 5, 2, C], E4, name=f"w2g{s}") for s in range(SPC)]
        w1pT_t = [wp.tile([C, CH], F32, name=f"w1pT{s}") for s in range(SPC)]
        b1g_t = [wp.tile([CH, 1], F32, name=f"b1g{s}") for s in range(SPC)]
        w2aT_t = [wp.tile([CH + 1, C], F32, name=f"w2aT{s}") for s in range(SPC)]
        spart = [spool.tile([C, 16], F32, name=f"spart{s}") for s in range(SPC)]
        sig = [spool.tile([C, 9], F32, name=f"sig{s}") for s in range(SPC)]
        red = [spool.tile([C, 4], F32, name=f"red{s}") for s in range(SPC)]
        x1sb = [spool.tile([C, 1], F32, name=f"x1sb{s}") for s in range(SPC)]
        a_aug = [spool.tile([CH + 1, 1], F32, name=f"aaug{s}") for s in range(SPC)]
        gprer = [spool.tile([1, C], F32, name=f"gprer{s}") for s in range(SPC)]
        gb = [spool.tile([C, C], F32, name=f"gb{s}") for s in range(SPC)]

        pdum = pv.tile([C, C], F32, name="pvs")
        for _ in range(34):
            nc.tensor.matmul(pdum[:, 0:C], wdum[:], zdum[:],
                             start=True, stop=True,
                             perf_mode=PM.DoubleRow)
        for s in range(SPC):
            # guard + zero rows of h1 (interior rewritten every sample)
            nc.vector.memset(h1[s][:, 0:2 * RS], 0)
            nc.vector.memset(h1[s][:, (SR - 2) * RS:], 0)
            nc.vector.memset(a_aug[s][CH:CH + 1, :], 1.0)

        # --- DMAs (SP queue) ---
        # plane-0 (conv input) bands first so PE starts asap; the C16
        # correction plane is only needed by the conv2 inject, so it
        # streams later.  Band k covers stored rows 33k .. 33k+36.
        def x_bands(s, ks, pl):
            for k in ks:
                a = RS * 33 * k
                b = min(GP, RS * (33 * k + 37))
                m = (a + b) // (2 * RS) * RS
                nc.sync.dma_start(xc[s][:, pl, a:m], xc_d[s, :, pl, a:m])
                nc.sync.dma_start(xc[s][:, pl, m:b], xc_d[s, :, pl, m:b])

        nc.scalar.dma_start(cw1_t[:], cw1_d)
        nc.sync.dma_start(xc[0][:, 0, 0:RS * 11], xc_d[0, :, 0, 0:RS * 11])
        nc.sync.dma_start(xc[0][:, 0, RS * 11:RS * 23],
                          xc_d[0, :, 0, RS * 11:RS * 23])
        nc.sync.dma_start(xc[0][:, 0, RS * 23:RS * 37],
                          xc_d[0, :, 0, RS * 23:RS * 37])
        for k in range(1, 4):
            a, b = RS * 33 * k, min(GP, RS * (33 * k + 37))
            nc.scalar.dma_start(xc[0][:, 0, a:b], xc_d[0, :, 0, a:b])
        nc.sync.dma_start(c1b_t[:], c1b_d)
        x_bands(1, range(4), 0)
        nc.sync.dma_start(i2_t[:], i2_d)
        nc.sync.dma_start(w2m_t[:], w2m_d)
        nc.sync.dma_start(w2s_t[:], w2s_d)
        nc.sync.dma_start(ones1_t[:], ones1_d)
        for s in range(SPC):
            nc.sync.dma_start(w1pT_t[s][:], w1pT_d[s])
            nc.sync.dma_start(b1g_t[s][:], b1g_d[s])
            nc.sync.dma_start(w2aT_t[s][:], w2aT_d[s])
        for s in range(SPC):
            for (a, b) in ((0, GP // 2), (GP // 2, GP)):
                nc.sync.dma_start(xc[s][:, 1, a:b], xc_d[s, :, 1, a:b])

        def conv_group(s, g, base_ap, pstride, wt, inj_base=None,
                       split=False):
            """10 DR tap matmuls (+2 injects) + wrap fix-ups -> 2-bank psum.

            base_ap: AP anchored at the image plane start.  With split=True
            each psum bank gets its own edge fix so the epilogue can drain
            bank 0 while the PE still fills bank 1 (shorter tail)."""
            pp = pc.tile([C, 2, 4, W], F32, name="pp")
            b0 = base_ap.offset
            # ff layout [C, side, h, row]: per-side 8 rows are contiguous
            ff = pf.tile([C, 2, 2, 4], F32, name="ff")
            pstr = pp[:].ap[0][0]

            def half(h):
                c = 2 * g + h
                for p in range(5):
                    ta, tb = PAIRS[p]
                    dy0, dx0 = ta // 3, ta % 3
                    dstr = 0 if tb is None else (
                        (tb // 3 - dy0) * RS + (tb % 3 - dx0))
                    rhs = base_ap.copy()
                    rhs.ap = V([[pstride, C], [dstr, 2], [1, 4 * W]])
                    rhs.offset = b0 + (4 * c + dy0 + 1) * RS + dx0 - 1
                    nc.tensor.matmul(pp[:, h], wt[:, p], rhs,
                                     start=(p == 0), stop=False,
                                     perf_mode=PM.DoubleRow,
                                     skip_group_check=True)
                if inj_base is not None:   # conv2: inject residual x
                    inj = inj_base.copy()
                    inj.ap = V([[2 * GP, C], [GP, 2], [1, 4 * W]])
                    inj.offset = inj_base.offset + (4 * c + 2) * RS
                    nc.tensor.matmul(pp[:, h], i2_t[:], inj,
                                     start=False, stop=False,
                                     perf_mode=PM.DoubleRow,
                                     skip_group_check=True)

            def fix(h, nrows=4):
                # wrap-around garbage at out cols 0/127: accumulate garbage
                # into contiguous psum F, then subtract from the edge cols.
                # col 0: taps (dy,0) = pair0 + single t6 (= wt[:,3,0]);
                # col 127: taps (dy,2) = pair2 + single t8 (= wt[:,3,1]).
                # nrows=8 covers both psum banks with one matmul pair.
                for side, (pair_p, single_sl, coff) in enumerate(
                        ((0, (3, 0), -1), (2, (3, 1), RS))):
                    if nrows == 4:
                        fo = ff[:, side, h]
                    else:
                        fo = ff[:, side].rearrange("p a b -> p (a b)")
                    src = base_ap.copy()
                    src.ap = V([[pstride, C], [RS, 2], [RS, nrows]])
                    src.offset = b0 + (8 * g + 4 * h + 1) * RS + coff
                    nc.tensor.matmul(fo, wt[:, pair_p], src,
                                     start=True, stop=False,
                                     perf_mode=PM.DoubleRow,
                                     skip_group_check=True)
                    src2 = base_ap.copy()
                    src2.ap = V([[pstride, C], [RS, nrows]])
                    src2.offset = b0 + (8 * g + 4 * h + 3) * RS + coff
                    nc.tensor.matmul(fo, wt[:, single_sl[0], single_sl[1]],
                                     src2, start=False, stop=True,
                                     skip_group_check=True)

            def merge(hs):
                # DVE may read only ONE operand from PSUM: stage F in SBUF
                # (fs mirrors ff's flat [side, h, row] layout)
                nh = len(hs)
                fs = spool.tile([C, 16], F32, name="fs", bufs=4)
                fdst = fs[:, 0:1].copy()
                fdst.ap = V([[16, C], [8, 2], [1, 4 * nh]])
                fdst.offset = fdst.offset + 4 * hs[0]
                fsrc = ff[:, 0, 0, 0:1].copy()
                fsrc.ap = V([[16, C], [8, 2], [1, 4 * nh]])
                fsrc.offset = fsrc.offset + 4 * hs[0]
                nc.vector.tensor_scalar(fdst, fsrc, 0.0, None, ALU.add)
                edge = pp[:].copy()
                edge.ap = V([[pstr, C], [512, nh], [128, 4], [W - 1, 2]])
                edge.offset = pp[:].offset + 512 * hs[0]
                fap = fs[:, 0:1].copy()
                fap.ap = V([[16, C], [4, nh], [1, 4], [8, 2]])
                fap.offset = fap.offset + 4 * hs[0]
                nc.vector.tensor_tensor(edge, edge, fap, op=ALU.subtract)

            if split:
                half(0); fix(0); merge([0])
                half(1); fix(1); merge([1])
            else:
                half(0); half(1); fix(0, nrows=8); merge([0, 1])
            return pp

        def conv1_group(s, g):
            pp = conv_group(s, g, xc[s][:, 0, 0:1], 2 * GP, cw1_t)
            a0 = (8 * g + 2) * RS
            out = h1[s][:, a0:a0 + 8 * RS].rearrange(
                "p (a b w) -> p a b w", a=2, b=4)
            if False and 0.0 <= prelu1 <= 1.0:
                # DVE epilogue relieves the Act queue at conv1 phase ends
                ct = ypool.tile([C, 2, 4, W], BF16, name="c1t")
                nc.vector.tensor_scalar(ct[:], pp[:], 1.0 / (SC * SC),
                                        c1b_t[:], ALU.mult, ALU.add)
                nc.vector.scalar_tensor_tensor(out, ct[:], prelu1, ct[:],
                                               op0=ALU.mult, op1=ALU.max)
                nc.vector.tensor_reduce(spart[s][:, g:g + 1],
                                        h1[s][:, a0:a0 + 8 * RS],
                                        axis=mybir.AxisListType.X,
                                        op=ALU.add)
            else:
                nc.scalar.activation(out, pp[:], AF.Prelu, bias=c1b_t[:],
                                     scale=1.0 / (SC * SC), alpha=prelu1,
                                     accum_out=spart[s][:, g:g + 1])

        def conv2_group(s, g, on_dve, split=False, tail=False):
            pp = conv_group(s, g, h1[s][:, 0:1], GP, w2g[s],
                            inj_base=xc[s][:, 0, 0:1], split=split)
            if tail:
                # split=True ordered the halves' fixes/merges separately:
                # drain each bank through Act + sync store as soon as ready
                for h in range(2):
                    yh = ypool.tile([C, 4, W], BF16, name="yh")
                    nc.scalar.activation(yh[:], pp[:, h], AF.Prelu,
                                         scale=1.0 / SC, alpha=prelu2)
                    nc.sync.dma_start(y_d[s, :, 8 * g + 4 * h:
                                          8 * g + 4 * h + 4, :], yh[:])
                return
            if split:
                # per-half epilogue + store on disjoint engines/queues:
                # drains bank 0 while the PE still fills bank 1
                yh = ypool.tile([C, 4, W], BF16, name="yh")
                nc.scalar.activation(yh[:], pp[:, 0], AF.Prelu,
                                     scale=1.0 / SC, alpha=prelu2)
                nc.sync.dma_start(y_d[s, :, 8 * g:8 * g + 4, :], yh[:])
                th = ypool.tile([C, 4, W], BF16, name="th")
                nc.vector.tensor_scalar(th[:], pp[:, 1], 1.0 / SC, None,
                                        ALU.mult)
                yh2 = ypool.tile([C, 4, W], BF16, name="yh2")
                nc.vector.scalar_tensor_tensor(yh2[:], th[:], prelu2, th[:],
                                               op0=ALU.mult, op1=ALU.max)
                nc.gpsimd.dma_start(y_d[s, :, 8 * g + 4:8 * g + 8, :],
                                    yh2[:])
                return
            yt = ypool.tile([C, 2, 4, W], BF16, name="yt")
            if on_dve:
                tt = ypool.tile([C, 2, 4, W], BF16, name="tt")
                nc.vector.tensor_scalar(tt[:], pp[:], 1.0 / SC, None, ALU.mult)
                nc.vector.scalar_tensor_tensor(yt[:], tt[:], prelu2, tt[:],
                                               op0=ALU.mult, op1=ALU.max)
            else:
                nc.scalar.activation(yt[:], pp[:], AF.Prelu,
                                     scale=1.0 / SC, alpha=prelu2)
            nc.sync.dma_start(
                y_d[s, :, 8 * g:8 * g + 8, :].rearrange(
                    "p (a b) w -> p a b w", a=2), yt[:])

        def pool_ssum(s, q):
            # pooling sum partial over data rows 32q..32q+31 on idle GPSIMD:
            # copy-to-trash with accum_out gives the free-axis sum
            a = (2 + 32 * q) * RS
            nc.gpsimd.tensor_scalar(strash[:], h1[s][:, a:a + 32 * RS],
                                    1.0, None, ALU.mult,
                                    accum_out=spart[s][:, q:q + 1])

        def stats_sigma(s):
            """strips + corners + sigma build (DVE), inclusion-exclusion."""
            hs = h1[s]
            X = mybir.AxisListType.X
            nc.vector.tensor_reduce(red[s][:, 0:1], spart[s][:], axis=X,
                                    op=ALU.add)                           # S (from Pool partials)
            nc.vector.tensor_reduce(red[s][:, 1:2], hs[:, 2 * RS:3 * RS],
                                    axis=X, op=ALU.add)                   # Rt
            nc.vector.tensor_reduce(red[s][:, 2:3],
                                    hs[:, (SR - 3) * RS:(SR - 2) * RS],
                                    axis=X, op=ALU.add)                   # Rb
            cl = hs[:, 0:1].copy()
            cl.ap = V([[GP, C], [RS, H]])
            cl.offset = cl.offset + 2 * RS
            nc.vector.tensor_reduce(red[s][:, 3:4], cl, axis=X, op=ALU.add)  # Cl
            crt = spool.tile([C, 1], F32, name=f"cr{s}")
            cr = hs[:, 0:1].copy()
            cr.ap = V([[GP, C], [RS, H]])
            cr.offset = cr.offset + 2 * RS + (W - 1)
            nc.vector.tensor_reduce(crt[:], cr, axis=X, op=ALU.add)          # Cr
            sg = sig[s]
            nc.vector.tensor_scalar(sg[:], z9_t[:], red[s][:, 0:1], None,
                                    ALU.add)
            nc.vector.tensor_scalar(sg[:, 0:3], sg[:, 0:3], red[s][:, 2:3],
                                    None, ALU.subtract)                   # dy=0: -Rb
            nc.vector.tensor_scalar(sg[:, 6:9], sg[:, 6:9], red[s][:, 1:2],
                                    None, ALU.subtract)                   # dy=2: -Rt
            for col0, which in ((0, crt[:]), (2, red[s][:, 3:4])):
                ap = sg[:, col0:col0 + 1].copy()
                ap.ap = V([[9, C], [3, 3]])
                nc.vector.tensor_scalar(ap, ap, which, None, ALU.subtract)
            # corners [(0,0),(0,127),(127,0),(127,127)]: strided fp8->f32 copy
            corn = spool.tile([C, 2, 2], F32, name=f"corn{s}")
            cap = hs[:, 0:1].copy()
            cap.ap = V([[GP, C], [(H - 1) * RS, 2], [W - 1, 2]])
            cap.offset = cap.offset + 2 * RS
            nc.vector.tensor_scalar(corn[:], cap, 0.0, None, ALU.add)
            cf = corn[:].rearrange("p a b -> p (a b)")
            for t, ci in ((8, 0), (6, 1), (2, 2), (0, 3)):
                nc.vector.tensor_scalar(sg[:, t:t + 1], sg[:, t:t + 1],
                                        cf[:, ci:ci + 1], None, ALU.add)

        def gate_mlp(s):
            t1 = pv.tile([C, C], F32, name="pvs")
            psx, psa = t1[:, 0:1], t1[0:CH, 1:2]
            for t in range(9):
                nc.tensor.matmul(psx, w2s_t[:, t], sig[s][:, t:t + 1],
                                 start=(t == 0), stop=(t == 8))
            nc.vector.tensor_scalar(x1sb[s][:], psx, 0.0, None, ALU.add)
            nc.tensor.matmul(psa, w1pT_t[s][:], x1sb[s][:],
                             start=True, stop=True)
            nc.vector.tensor_scalar(a_aug[s][0:CH, :], psa, b1g_t[s][:],
                                    0.0, ALU.add, ALU.max)
            psg = pv.tile([C, C], F32, name="pvs")[0:1, :]
            nc.tensor.matmul(psg, a_aug[s][:], w2aT_t[s][:],
                             start=True, stop=True)
            nc.vector.tensor_scalar(gprer[s][:], psg, 0.0, None, ALU.add)
            psb = pv.tile([C, C], F32, name="pvs")
            nc.tensor.matmul(psb[:], ones1_t[:], gprer[s][:],
                             start=True, stop=True)
            nc.scalar.activation(gb[s][:], psb[:], AF.Sigmoid)

        def fold_w2(s):
            def bc(shape_dims):
                ap = gb[s][:, 0:1].copy()
                ap.ap = V([[C, C]] + shape_dims)
                return ap
            nc.gpsimd.tensor_tensor(w2g[s][:], w2m_t[:],
                                    bc([[0, 5], [0, 2], [1, C]]), op=ALU.mult)

        # ---------------- emission schedule ----------------
        for g in range(16):
            conv1_group(0, g)
        stats_sigma(0)
        for g in range(8):
            conv1_group(1, g)
        gate_mlp(0)
        for g in range(8, 12):
            conv1_group(1, g)
        fold_w2(0)
        for g in range(12, 16):
            conv1_group(1, g)
        stats_sigma(1)
        for g in range(8):
            conv2_group(0, g, on_dve=False)
        gate_mlp(1)
        for g in range(8, 12):
            conv2_group(0, g, on_dve=False)
        fold_w2(1)
        for g in range(12, 16):
            conv2_group(0, g, on_dve=False)
        for g in range(15):
            conv2_group(1, g, on_dve=False)
        conv2_group(1, 15, on_dve=False, split=True, tail=True)

    nc.compile()
    return nc


_CACHE = {}


def _get_program(prelu1, prelu2):
    key = (float(prelu1), float(prelu2))
    if key not in _CACHE:
        _CACHE[key] = _build(*key)
    return _CACHE[key]


def _prep(x, intensity, conv1_w, conv1_b, prelu1, conv2_w, conv2_b,
          aW1, ab1, aW2, ab2, prelu2):
    x = np.asarray(x, np.float32)
    idx = np.asarray(intensity).astype(np.int64) - 1
    conv1_w = np.asarray(conv1_w, np.float32)
    conv1_b = np.asarray(conv1_b, np.float32)
    conv2_w = np.asarray(conv2_w, np.float32)
    conv2_b = np.asarray(conv2_b, np.float32)
    aW1 = np.asarray(aW1, np.float32)
    ab1 = np.asarray(ab1, np.float32)
    aW2 = np.asarray(aW2, np.float32)
    ab2 = np.asarray(ab2, np.float32)
    assert not np.any(conv2_b), "conv2 bias folding not implemented"

    # stored planes: guard row, zero row, 128 data rows, zero row, guard row
    xpad = np.zeros((N, C, SR, RS), np.float32)
    xpad[:, :, 2:H + 2, :] = x * SC
    x16 = xpad.astype(E4NP)
    c16 = (xpad - x16.astype(np.float32)).astype(E4NP)
    xc = np.stack([x16, c16], axis=2).reshape(N, C, 2, GP)

    wtap1 = conv1_w.transpose(1, 2, 3, 0).reshape(C, 9, C)  # [i, t, o]
    wtap2 = conv2_w.transpose(1, 2, 3, 0).reshape(C, 9, C)

    def pair_pack(wtap, dtype):
        out = np.zeros((C, 5, 2, C), np.float32)
        for p, (ta, tb) in enumerate(PAIRS):
            out[:, p, 0] = wtap[:, ta]
            if tb is not None:
                out[:, p, 1] = wtap[:, tb]
        return np.ascontiguousarray((out * SC).astype(dtype))

    cw1 = pair_pack(wtap1, E4NP)
    w2m = pair_pack(wtap2, BF)
    w2s = np.ascontiguousarray(wtap2)
    i2 = np.zeros((C, 2, C), np.float32)
    i2[:, 0] = np.eye(C)
    i2[:, 1] = np.eye(C)
    i2 = i2.astype(E4NP)

    w1pT = np.ascontiguousarray(
        (aW1[idx] / (SC * H * W)).transpose(0, 2, 1))     # [N, C, CH]
    b1g = np.ascontiguousarray(ab1[idx])[:, :, None]      # [N, CH, 1]
    w2aT = np.concatenate(
        [aW2[idx].transpose(0, 2, 1), ab2[idx][:, None, :]], axis=1)

    nc = _get_program(float(prelu1), float(prelu2))

    in_maps = []
    for i in range(NCORES):
        sl = slice(i * SPC, (i + 1) * SPC)
        in_maps.append(dict(
            xc=xc[sl], cw1=cw1, w2m=w2m, w2s=w2s, i2=i2,
            ones1=np.ones((1, C), np.float32),
            c1b=conv1_b[:, None],
            w1pT=np.ascontiguousarray(w1pT[sl]),
            b1g=np.ascontiguousarray(b1g[sl]),
            w2aT=np.ascontiguousarray(w2aT[sl])))
    return nc, in_maps


def kernel(**inputs):
    import time
    from concourse.bass_utils import run_bass_kernel_spmd

    nc, in_maps = _prep(**inputs)
    res = None
    for attempt, pause in enumerate((0, 15, 60, 120)):
        if pause:
            time.sleep(pause)
        try:
            res = run_bass_kernel_spmd(nc, in_maps,
                                       core_ids=list(range(NCORES)))
            break
        except Exception:
            # transient NRT_EXEC_UNIT_UNRECOVERABLE (wedged core); retry
            if attempt == 3:
                raise
    return np.concatenate(
        [r["y"].astype(np.float32) for r in res.results], axis=0)


# revision 65
# speedup vs baseline: 1.0510x; 1.0243x over previous
"""Trainium2 Bass kernel for nn_DomainAdaption (conv-conv-MoE-gated-residual).

Data-parallel over batch: 16 samples -> 8 NeuronCores, 2 samples/core.

Everything heavy runs through fp8e4 DoubleRow matmuls (0.5 cyc/row, 2x128
contraction per instruction).  Images are stored with row stride 128 (NO
column padding) so each 4-row conv window is one contiguous 512-element
run -- the DR moving operand is then a clean [128, 2, 512] AP whose pair
dim selects two conv taps (pair strides must be 0, 2 or >=128: stride 1
wedges the PE, hence pairs (t0,t3)(t1,t4)(t2,t5)(t6,t8)(t7,-)).
Horizontal padding is emulated: the wrap-around garbage that taps dx=0 /
dx=2 read at columns 0/127 is recomputed into a tiny contiguous psum tile
by 4 fix-up matmuls per group, staged to SBUF (DVE may read only one PSUM
operand), and subtracted from the psum edge columns by one DVE op.
Vertical padding is real (zero rows), plus guard rows front/back.

Per sample:
  conv1: 5 DR pair-matmuls per [4x128] chunk over fp8(16*x); ScalarE Prelu
         epilogue (scale 1/256) writes h1 fp8 + pooling partials
         (accum_out).
  gate EARLY (before conv2): mean(conv2(h1)) is computed exactly from 9
         reduced h1 vectors (sum S, edge strips, corners) via
         inclusion-exclusion over the conv window, then 9 tiny f32
         stat-matmuls + adapter MLP + a broadcast matmul + sigmoid
         (all on-device; hides under the other sample's conv phase).
  conv2: the gate is folded into the fp8 conv2 weights on-device
         (w2g = fp8(16*w2*g[o]), one GPSIMD multiply against a
         PE-broadcast gate plane) and the residual x is injected INTO the
         conv2 PSUM as a DR pair (I @ fp8(16x) + I @ fp8 correction), so
         a single Prelu epilogue (scale 1/16) emits
         y = prelu(g*conv2(h1) + x) in bf16 directly -- h2 is never
         materialized and there is no separate residual pass.

PE order c1(s0), c1(s1), c2(s0), c2(s1) with stats/gate/fold of each
sample emitted mid-phase of the other sample, so the PE never waits on
the gate chain.  Dummy warm-up matmuls during the initial DMA fill keep
the PE p-state ramp off the critical path.  285us (baseline) -> 98.7us
measured on TimelineSim; hardware rel err 9.9e-3 vs the fp32 reference.
"""
import sys

if "/opt/trn_rl_repo" not in sys.path:
    sys.path.insert(0, "/opt/trn_rl_repo")

import numpy as np
import ml_dtypes

N, C, H, W = 16, 128, 128, 128
CH = 32
NCORES = 8
SPC = N // NCORES          # samples per core
RS = W                     # stored row stride
SR = H + 4                 # stored rows: guard, zero, 128 data, zero, guard
GP = SR * RS               # elements per stored plane (16896)
SC = 16.0                  # fp8 scale for x and conv weights
BF = ml_dtypes.bfloat16
E4NP = ml_dtypes.float8_e4m3fn

# DoubleRow tap pairs: (tap_a, tap_b); taps are t = 3*dy + dx.
PAIRS = [(0, 3), (1, 4), (2, 5), (6, 8), (7, None)]


def _build(prelu1: float, prelu2: float):
    import concourse.mybir as mybir
    import concourse.tile as tile
    from concourse import bacc
    import bass_rust

    F32 = mybir.dt.float32
    F32R = mybir.dt.float32r
    BF16 = mybir.dt.bfloat16
    E4 = mybir.dt.float8e4
    AF = mybir.ActivationFunctionType
    ALU = mybir.AluOpType
    PM = mybir.MatmulPerfMode
    V = bass_rust.VecI64Pair

    nc = bacc.Bacc("TRN2", target_bir_lowering=False, debug=False,
                   num_devices=NCORES)

    xc_d = nc.dram_tensor("xc", [SPC, C, 2, GP], E4, kind="ExternalInput").ap()
    cw1_d = nc.dram_tensor("cw1", [C, 5, 2, C], E4, kind="ExternalInput").ap()
    w2m_d = nc.dram_tensor("w2m", [C, 5, 2, C], BF16, kind="ExternalInput").ap()
    w2s_d = nc.dram_tensor("w2s", [C, 9, C], F32, kind="ExternalInput").ap()
    i2_d = nc.dram_tensor("i2", [C, 2, C], E4, kind="ExternalInput").ap()
    ones1_d = nc.dram_tensor("ones1", [1, C], F32, kind="ExternalInput").ap()
    c1b_d = nc.dram_tensor("c1b", [C, 1], F32, kind="ExternalInput").ap()
    w1pT_d = nc.dram_tensor("w1pT", [SPC, C, CH], F32, kind="ExternalInput").ap()
    b1g_d = nc.dram_tensor("b1g", [SPC, CH, 1], F32, kind="ExternalInput").ap()
    w2aT_d = nc.dram_tensor("w2aT", [SPC, CH + 1, C], F32, kind="ExternalInput").ap()
    y_d = nc.dram_tensor("y", [SPC, C, H, W], BF16, kind="ExternalOutput").ap()

    with tile.TileContext(nc) as tc, (
        tc.tile_pool(name="wp", bufs=1)) as wp, (
        tc.tile_pool(name="xp", bufs=1)) as xp, (
        tc.tile_pool(name="hp", bufs=1)) as hpool, (
        tc.tile_pool(name="sp", bufs=1)) as spool, (
        tc.tile_pool(name="yp", bufs=4)) as ypool, (
        tc.tile_pool(name="pc", bufs=3, space="PSUM")) as pc, (
        tc.tile_pool(name="pv", bufs=1, space="PSUM")) as pv, (
        tc.tile_pool(name="pf", bufs=1, space="PSUM")) as pf:

        # --- static weights / constants ---
        cw1_t = wp.tile([C, 5, 2, C], E4, name="cw1t")
        w2m_t = wp.tile([C, 5, 2, C], BF16, name="w2mt")
        w2s_t = wp.tile([C, 9, C], F32, name="w2st")
        i2_t = wp.tile([C, 2, C], E4, name="i2t")
        ones1_t = wp.tile([1, C], F32, name="ones1t")
        c1b_t = wp.tile([C, 1], F32, name="c1bt")
        z9_t = wp.tile([C, 9], F32, name="z9t")
        strash = wp.tile([C, 32 * RS], E4, name="strash")
        wdum = wp.tile([C, 2, C], E4, name="wdum")
        zdum = wp.tile([C, 2, C], E4, name="zdum")
        nc.vector.memset(wdum[:], 0)
        nc.vector.memset(zdum[:], 0)
        nc.vector.memset(z9_t[:], 0)

        # per-sample tiles
        xc = [xp.tile([C, 2, GP], E4, name=f"xc{s}") for s in range(SPC)]
        h1 = [hpool.tile([C, GP], E4, name=f"h1_{s}") for s in range(SPC)]
        w2g = [wp.tile([C, 5, 2, C], E4, name=f"w2g{s}") for s in range(SPC)]
        w1pT_t = [wp.tile([C, CH], F32, name=f"w1pT{s}") for s in range(SPC)]
        b1g_t = [wp.tile([CH, 1], F32, name=f"b1g{s}") for s in range(SPC)]
        w2aT_t = [wp.tile([CH + 1, C], F32, name=f"w2aT{s}") for s in range(SPC)]
        spart = [spool.tile([C, 16], F32, name=f"spart{s}") for s in range(SPC)]
        sig = [spool.tile([C, 9], F32, name=f"sig{s}") for s in range(SPC)]
        red = [spool.tile([C, 4], F32, name=f"red{s}") for s in range(SPC)]
        x1sb = [spool.tile([C, 1], F32, name=f"x1sb{s}") for s in range(SPC)]
        a_aug = [spool.tile([CH + 1, 1], F32, name=f"aaug{s}") for s in range(SPC)]
        gprer = [spool.tile([1, C], F32, name=f"gprer{s}") for s in range(SPC)]
        gb = [spool.tile([C, C], F32, name=f"gb{s}") for s in range(SPC)]

        pdum = pv.tile([C, C], F32, name="pvs")
        for _ in range(34):
            nc.tensor.matmul(pdum[:, 0:C], wdum[:], zdum[:],
                             start=True, stop=True,
                             perf_mode=PM.DoubleRow)
        for s in range(SPC):
            # guard + zero rows of h1 (interior rewritten every sample)
            nc.vector.memset(h1[s][:, 0:2 * RS], 0)
            nc.vector.memset(h1[s][:, (SR - 2) * RS:], 0)
            nc.vector.memset(a_aug[s][CH:CH + 1, :], 1.0)

        # --- DMAs (SP queue) ---
        # plane-0 (conv input) bands first so PE starts asap; the C16
        # correction plane is only needed by the conv2 inject, so it
        # streams later.  Band k covers stored rows 33k .. 33k+36.
        def x_bands(s, ks, pl):
            for k in ks:
                a = RS * 33 * k
                b = min(GP, RS * (33 * k + 37))
                m = (a + b) // (2 * RS) * RS
                nc.sync.dma_start(xc[s][:, pl, a:m], xc_d[s, :, pl, a:m])
                nc.sync.dma_start(xc[s][:, pl, m:b], xc_d[s, :, pl, m:b])

        nc.scalar.dma_start(cw1_t[:], cw1_d)
        for (a, b) in ((0, 13), (13, 46), (46, 90), (90, SR)):
            nc.sync.dma_start(xc[0][:, 0, RS * a:RS * b],
                              xc_d[0, :, 0, RS * a:RS * b])
        nc.sync.dma_start(c1b_t[:], c1b_d)
        for (a, b) in ((0, 34), (34, 67), (67, 100), (100, SR)):
            nc.sync.dma_start(xc[1][:, 0, RS * a:RS * b],
                              xc_d[1, :, 0, RS * a:RS * b])
        nc.sync.dma_start(i2_t[:], i2_d)
        nc.sync.dma_start(w2m_t[:], w2m_d)
        nc.sync.dma_start(w2s_t[:], w2s_d)
        nc.sync.dma_start(ones1_t[:], ones1_d)
        for s in range(SPC):
            nc.sync.dma_start(w1pT_t[s][:], w1pT_d[s])
            nc.sync.dma_start(b1g_t[s][:], b1g_d[s])
            nc.sync.dma_start(w2aT_t[s][:], w2aT_d[s])
        for s in range(SPC):
            for (a, b) in ((0, GP // 2), (GP // 2, GP)):
                nc.sync.dma_start(xc[s][:, 1, a:b], xc_d[s, :, 1, a:b])

        def conv_group(s, g, base_ap, pstride, wt, inj_base=None,
                       split=False):
            """10 DR tap matmuls (+2 injects) + wrap fix-ups -> 2-bank psum.

            base_ap: AP anchored at the image plane start.  With split=True
            each psum bank gets its own edge fix so the epilogue can drain
            bank 0 while the PE still fills bank 1 (shorter tail)."""
            pp = pc.tile([C, 2, 4, W], F32, name="pp")
            b0 = base_ap.offset
            # ff layout [C, side, h, row]: per-side 8 rows are contiguous
            ff = pf.tile([C, 2, 2, 4], F32, name="ff")
            pstr = pp[:].ap[0][0]

            def half(h):
                c = 2 * g + h
                for p in range(5):
                    ta, tb = PAIRS[p]
                    dy0, dx0 = ta // 3, ta % 3
                    dstr = 0 if tb is None else (
                        (tb // 3 - dy0) * RS + (tb % 3 - dx0))
                    rhs = base_ap.copy()
                    rhs.ap = V([[pstride, C], [dstr, 2], [1, 4 * W]])
                    rhs.offset = b0 + (4 * c + dy0 + 1) * RS + dx0 - 1
                    nc.tensor.matmul(pp[:, h], wt[:, p], rhs,
                                     start=(p == 0), stop=False,
                                     perf_mode=PM.DoubleRow,
                                     skip_group_check=True)
                if inj_base is not None:   # conv2: inject residual x
                    inj = inj_base.copy()
                    inj.ap = V([[2 * GP, C], [GP, 2], [1, 4 * W]])
                    inj.offset = inj_base.offset + (4 * c + 2) * RS
                    nc.tensor.matmul(pp[:, h], i2_t[:], inj,
                                     start=False, stop=False,
                                     perf_mode=PM.DoubleRow,
                                     skip_group_check=True)

            def fix(h, nrows=4):
                # wrap-around garbage at out cols 0/127: accumulate garbage
                # into contiguous psum F, then subtract from the edge cols.
                # col 0: taps (dy,0) = pair0 + single t6 (= wt[:,3,0]);
                # col 127: taps (dy,2) = pair2 + single t8 (= wt[:,3,1]).
                # nrows=8 covers both psum banks with one matmul pair.
                for side, (pair_p, single_sl, coff) in enumerate(
                        ((0, (3, 0), -1), (2, (3, 1), RS))):
                    if nrows == 4:
                        fo = ff[:, side, h]
                    else:
                        fo = ff[:, side].rearrange("p a b -> p (a b)")
                    src = base_ap.copy()
                    src.ap = V([[pstride, C], [RS, 2], [RS, nrows]])
                    src.offset = b0 + (8 * g + 4 * h + 1) * RS + coff
                    nc.tensor.matmul(fo, wt[:, pair_p], src,
                                     start=True, stop=False,
                                     perf_mode=PM.DoubleRow,
                                     skip_group_check=True)
                    src2 = base_ap.copy()
                    src2.ap = V([[pstride, C], [RS, nrows]])
                    src2.offset = b0 + (8 * g + 4 * h + 3) * RS + coff
                    nc.tensor.matmul(fo, wt[:, single_sl[0], single_sl[1]],
                                     src2, start=False, stop=True,
                                     skip_group_check=True)

            def merge(hs):
                # DVE may read only ONE operand from PSUM: stage F in SBUF
                # (fs mirrors ff's flat [side, h, row] layout)
                nh = len(hs)
                fs = spool.tile([C, 16], F32, name="fs", bufs=4)
                fdst = fs[:, 0:1].copy()
                fdst.ap = V([[16, C], [8, 2], [1, 4 * nh]])
                fdst.offset = fdst.offset + 4 * hs[0]
                fsrc = ff[:, 0, 0, 0:1].copy()
                fsrc.ap = V([[16, C], [8, 2], [1, 4 * nh]])
                fsrc.offset = fsrc.offset + 4 * hs[0]
                nc.vector.tensor_scalar(fdst, fsrc, 0.0, None, ALU.add)
                edge = pp[:].copy()
                edge.ap = V([[pstr, C], [512, nh], [128, 4], [W - 1, 2]])
                edge.offset = pp[:].offset + 512 * hs[0]
                fap = fs[:, 0:1].copy()
                fap.ap = V([[16, C], [4, nh], [1, 4], [8, 2]])
                fap.offset = fap.offset + 4 * hs[0]
                nc.vector.tensor_tensor(edge, edge, fap, op=ALU.subtract)

            if split:
                half(0); fix(0); merge([0])
                half(1); fix(1); merge([1])
            else:
                half(0); half(1); fix(0, nrows=8); merge([0, 1])
            return pp

        def conv1_group(s, g):
            pp = conv_group(s, g, xc[s][:, 0, 0:1], 2 * GP, cw1_t)
            a0 = (8 * g + 2) * RS
            out = h1[s][:, a0:a0 + 8 * RS].rearrange(
                "p (a b w) -> p a b w", a=2, b=4)
            nc.scalar.activation(out, pp[:], AF.Prelu, bias=c1b_t[:],
                                 scale=1.0 / (SC * SC), alpha=prelu1,
                                 accum_out=spart[s][:, g:g + 1])

        def conv2_group(s, g, on_dve, split=False, tail=False):
            pp = conv_group(s, g, h1[s][:, 0:1], GP, w2g[s],
                            inj_base=xc[s][:, 0, 0:1], split=split)
            if tail:
                # split=True ordered the halves' fixes/merges separately:
                # drain each bank through Act + sync store as soon as ready
                for h in range(2):
                    yh = ypool.tile([C, 4, W], BF16, name="yh")
                    nc.scalar.activation(yh[:], pp[:, h], AF.Prelu,
                                         scale=1.0 / SC, alpha=prelu2)
                    nc.sync.dma_start(y_d[s, :, 8 * g + 4 * h:
                                          8 * g + 4 * h + 4, :], yh[:])
                return
            if split:
                # per-half epilogue + store on disjoint engines/queues:
                # drains bank 0 while the PE still fills bank 1
                yh = ypool.tile([C, 4, W], BF16, name="yh")
                nc.scalar.activation(yh[:], pp[:, 0], AF.Prelu,
                                     scale=1.0 / SC, alpha=prelu2)
                nc.sync.dma_start(y_d[s, :, 8 * g:8 * g + 4, :], yh[:])
                th = ypool.tile([C, 4, W], BF16, name="th")
                nc.vector.tensor_scalar(th[:], pp[:, 1], 1.0 / SC, None,
                                        ALU.mult)
                yh2 = ypool.tile([C, 4, W], BF16, name="yh2")
                nc.vector.scalar_tensor_tensor(yh2[:], th[:], prelu2, th[:],
                                               op0=ALU.mult, op1=ALU.max)
                nc.gpsimd.dma_start(y_d[s, :, 8 * g + 4:8 * g + 8, :],
                                    yh2[:])
                return
            yt = ypool.tile([C, 2, 4, W], BF16, name="yt")
            if on_dve:
                tt = ypool.tile([C, 2, 4, W], BF16, name="tt")
                nc.vector.tensor_scalar(tt[:], pp[:], 1.0 / SC, None, ALU.mult)
                nc.vector.scalar_tensor_tensor(yt[:], tt[:], prelu2, tt[:],
                                               op0=ALU.mult, op1=ALU.max)
            else:
                nc.scalar.activation(yt[:], pp[:], AF.Prelu,
                                     scale=1.0 / SC, alpha=prelu2)
            nc.sync.dma_start(
                y_d[s, :, 8 * g:8 * g + 8, :].rearrange(
                    "p (a b) w -> p a b w", a=2), yt[:])

        def pool_ssum(s, q):
            # pooling sum partial over data rows 32q..32q+31 on idle GPSIMD:
            # copy-to-trash with accum_out gives the free-axis sum
            a = (2 + 32 * q) * RS
            nc.gpsimd.tensor_scalar(strash[:], h1[s][:, a:a + 32 * RS],
                                    1.0, None, ALU.mult,
                                    accum_out=spart[s][:, q:q + 1])

        def stats_sigma(s):
            """strips + corners + sigma build (DVE), inclusion-exclusion."""
            hs = h1[s]
            X = mybir.AxisListType.X
            nc.vector.tensor_reduce(red[s][:, 0:1], spart[s][:], axis=X,
                                    op=ALU.add)                           # S (from Pool partials)
            nc.vector.tensor_reduce(red[s][:, 1:2], hs[:, 2 * RS:3 * RS],
                                    axis=X, op=ALU.add)                   # Rt
            nc.vector.tensor_reduce(red[s][:, 2:3],
                                    hs[:, (SR - 3) * RS:(SR - 2) * RS],
                                    axis=X, op=ALU.add)                   # Rb
            cl = hs[:, 0:1].copy()
            cl.ap = V([[GP, C], [RS, H]])
            cl.offset = cl.offset + 2 * RS
            nc.vector.tensor_reduce(red[s][:, 3:4], cl, axis=X, op=ALU.add)  # Cl
            crt = spool.tile([C, 1], F32, name=f"cr{s}")
            cr = hs[:, 0:1].copy()
            cr.ap = V([[GP, C], [RS, H]])
            cr.offset = cr.offset + 2 * RS + (W - 1)
            nc.vector.tensor_reduce(crt[:], cr, axis=X, op=ALU.add)          # Cr
            sg = sig[s]
            nc.vector.tensor_scalar(sg[:], z9_t[:], red[s][:, 0:1], None,
                                    ALU.add)
            nc.vector.tensor_scalar(sg[:, 0:3], sg[:, 0:3], red[s][:, 2:3],
                                    None, ALU.subtract)                   # dy=0: -Rb
            nc.vector.tensor_scalar(sg[:, 6:9], sg[:, 6:9], red[s][:, 1:2],
                                    None, ALU.subtract)                   # dy=2: -Rt
            for col0, which in ((0, crt[:]), (2, red[s][:, 3:4])):
                ap = sg[:, col0:col0 + 1].copy()
                ap.ap = V([[9, C], [3, 3]])
                nc.vector.tensor_scalar(ap, ap, which, None, ALU.subtract)
            # corners [(0,0),(0,127),(127,0),(127,127)]: strided fp8->f32 copy
            corn = spool.tile([C, 2, 2], F32, name=f"corn{s}")
            cap = hs[:, 0:1].copy()
            cap.ap = V([[GP, C], [(H - 1) * RS, 2], [W - 1, 2]])
            cap.offset = cap.offset + 2 * RS
            nc.vector.tensor_scalar(corn[:], cap, 0.0, None, ALU.add)
            cf = corn[:].rearrange("p a b -> p (a b)")
            for t, ci in ((8, 0), (6, 1), (2, 2), (0, 3)):
                nc.vector.tensor_scalar(sg[:, t:t + 1], sg[:, t:t + 1],
                                        cf[:, ci:ci + 1], None, ALU.add)

        def gate_mlp(s):
            t1 = pv.tile([C, C], F32, name="pvs")
            psx, psa = t1[:, 0:1], t1[0:CH, 1:2]
            for t in range(9):
                nc.tensor.matmul(psx, w2s_t[:, t], sig[s][:, t:t + 1],
                                 start=(t == 0), stop=(t == 8))
            nc.vector.tensor_scalar(x1sb[s][:], psx, 0.0, None, ALU.add)
            nc.tensor.matmul(psa, w1pT_t[s][:], x1sb[s][:],
                             start=True, stop=True)
            nc.vector.tensor_scalar(a_aug[s][0:CH, :], psa, b1g_t[s][:],
                                    0.0, ALU.add, ALU.max)
            psg = pv.tile([C, C], F32, name="pvs")[0:1, :]
            nc.tensor.matmul(psg, a_aug[s][:], w2aT_t[s][:],
                             start=True, stop=True)
            nc.vector.tensor_scalar(gprer[s][:], psg, 0.0, None, ALU.add)
            psb = pv.tile([C, C], F32, name="pvs")
            nc.tensor.matmul(psb[:], ones1_t[:], gprer[s][:],
                             start=True, stop=True)
            nc.scalar.activation(gb[s][:], psb[:], AF.Sigmoid)

        def fold_w2(s):
            def bc(shape_dims):
                ap = gb[s][:, 0:1].copy()
                ap.ap = V([[C, C]] + shape_dims)
                return ap
            nc.gpsimd.tensor_tensor(w2g[s][:], w2m_t[:],
                                    bc([[0, 5], [0, 2], [1, C]]), op=ALU.mult)

        # ---------------- emission schedule ----------------
        for g in range(16):
            conv1_group(0, g)
        stats_sigma(0)
        for g in range(8):
            conv1_group(1, g)
        gate_mlp(0)
        for g in range(8, 12):
            conv1_group(1, g)
        fold_w2(0)
        for g in range(12, 16):
            conv1_group(1, g)
        stats_sigma(1)
        for g in range(8):
            conv2_group(0, g, on_dve=False)
        gate_mlp(1)
        for g in range(8, 12):
            conv2_group(0, g, on_dve=False)
        fold_w2(1)
        for g in range(12, 16):
            conv2_group(0, g, on_dve=False)
        for g in range(15):
            conv2_group(1, g, on_dve=False)
        conv2_group(1, 15, on_dve=False, split=True, tail=True)

    nc.compile()
    return nc


_CACHE = {}


def _get_program(prelu1, prelu2):
    key = (float(prelu1), float(prelu2))
    if key not in _CACHE:
        _CACHE[key] = _build(*key)
    return _CACHE[key]


def _prep(x, intensity, conv1_w, conv1_b, prelu1, conv2_w, conv2_b,
          aW1, ab1, aW2, ab2, prelu2):
    x = np.asarray(x, np.float32)
    idx = np.asarray(intensity).astype(np.int64) - 1
    conv1_w = np.asarray(conv1_w, np.float32)
    conv1_b = np.asarray(conv1_b, np.float32)
    conv2_w = np.asarray(conv2_w, np.float32)
    conv2_b = np.asarray(conv2_b, np.float32)
    aW1 = np.asarray(aW1, np.float32)
    ab1 = np.asarray(ab1, np.float32)
    aW2 = np.asarray(aW2, np.float32)
    ab2 = np.asarray(ab2, np.float32)
    assert not np.any(conv2_b), "conv2 bias folding not implemented"

    # stored planes: guard row, zero row, 128 data rows, zero row, guard row
    xpad = np.zeros((N, C, SR, RS), np.float32)
    xpad[:, :, 2:H + 2, :] = x * SC
    x16 = xpad.astype(E4NP)
    c16 = (xpad - x16.astype(np.float32)).astype(E4NP)
    xc = np.stack([x16, c16], axis=2).reshape(N, C, 2, GP)

    wtap1 = conv1_w.transpose(1, 2, 3, 0).reshape(C, 9, C)  # [i, t, o]
    wtap2 = conv2_w.transpose(1, 2, 3, 0).reshape(C, 9, C)

    def pair_pack(wtap, dtype):
        out = np.zeros((C, 5, 2, C), np.float32)
        for p, (ta, tb) in enumerate(PAIRS):
            out[:, p, 0] = wtap[:, ta]
            if tb is not None:
                out[:, p, 1] = wtap[:, tb]
        return np.ascontiguousarray((out * SC).astype(dtype))

    cw1 = pair_pack(wtap1, E4NP)
    w2m = pair_pack(wtap2, BF)
    w2s = np.ascontiguousarray(wtap2)
    i2 = np.zeros((C, 2, C), np.float32)
    i2[:, 0] = np.eye(C)
    i2[:, 1] = np.eye(C)
    i2 = i2.astype(E4NP)

    w1pT = np.ascontiguousarray(
        (aW1[idx] / (SC * H * W)).transpose(0, 2, 1))     # [N, C, CH]
    b1g = np.ascontiguousarray(ab1[idx])[:, :, None]      # [N, CH, 1]
    w2aT = np.concatenate(
        [aW2[idx].transpose(0, 2, 1), ab2[idx][:, None, :]], axis=1)

    nc = _get_program(float(prelu1), float(prelu2))

    in_maps = []
    for i in range(NCORES):
        sl = slice(i * SPC, (i + 1) * SPC)
        in_maps.append(dict(
            xc=xc[sl], cw1=cw1, w2m=w2m, w2s=w2s, i2=i2,
            ones1=np.ones((1, C), np.float32),
            c1b=conv1_b[:, None],
            w1pT=np.ascontiguousarray(w1pT[sl]),
            b1g=np.ascontiguousarray(b1g[sl]),
            w2aT=np.ascontiguousarray(w2aT[sl])))
    return nc, in_maps


def kernel(**inputs):
    import time
    from concourse.bass_utils import run_bass_kernel_spmd

    nc, in_maps = _prep(**inputs)
    res = None
    for attempt, pause in enumerate((0, 15, 60, 120)):
        if pause:
            time.sleep(pause)
        try:
            res = run_bass_kernel_spmd(nc, in_maps,
                                       core_ids=list(range(NCORES)))
            break
        except Exception:
            # transient NRT_EXEC_UNIT_UNRECOVERABLE (wedged core); retry
            if attempt == 3:
                raise
    return np.concatenate(
        [r["y"].astype(np.float32) for r in res.results], axis=0)


# revision 69
# speedup vs baseline: 1.0598x; 1.0084x over previous
"""Trainium2 Bass kernel for nn_DomainAdaption (conv-conv-MoE-gated-residual).

Data-parallel over batch: 16 samples -> 8 NeuronCores, 2 samples/core.

Everything heavy runs through fp8e4 DoubleRow matmuls (0.5 cyc/row, 2x128
contraction per instruction).  Images are stored with row stride 128 (NO
column padding) so each 4-row conv window is one contiguous 512-element
run -- the DR moving operand is then a clean [128, 2, 512] AP whose pair
dim selects two conv taps (pair strides must be 0, 2 or >=128: stride 1
wedges the PE, hence pairs (t0,t3)(t1,t4)(t2,t5)(t6,t8)(t7,-)).
Horizontal padding is emulated: the wrap-around garbage that taps dx=0 /
dx=2 read at columns 0/127 is recomputed into a tiny contiguous psum tile
by 4 fix-up matmuls per group, staged to SBUF (DVE may read only one PSUM
operand), and subtracted from the psum edge columns by one DVE op.
Vertical padding is real (zero rows), plus guard rows front/back.

Per sample:
  conv1: 5 DR pair-matmuls per [4x128] chunk over fp8(16*x); ScalarE Prelu
         epilogue (scale 1/256) writes h1 fp8 + pooling partials
         (accum_out).
  gate EARLY (before conv2): mean(conv2(h1)) is computed exactly from 9
         reduced h1 vectors (sum S, edge strips, corners) via
         inclusion-exclusion over the conv window, then 9 tiny f32
         stat-matmuls + adapter MLP + a broadcast matmul + sigmoid
         (all on-device; hides under the other sample's conv phase).
  conv2: the gate is folded into the fp8 conv2 weights on-device
         (w2g = fp8(16*w2*g[o]), one GPSIMD multiply against a
         PE-broadcast gate plane) and the residual x is injected INTO the
         conv2 PSUM as a DR pair (I @ fp8(16x) + I @ fp8 correction), so
         a single Prelu epilogue (scale 1/16) emits
         y = prelu(g*conv2(h1) + x) in bf16 directly -- h2 is never
         materialized and there is no separate residual pass.

PE order c1(s0), c1(s1), c2(s0), c2(s1) with stats/gate/fold of each
sample emitted mid-phase of the other sample, so the PE never waits on
the gate chain.  Dummy warm-up matmuls during the initial DMA fill keep
the PE p-state ramp off the critical path.  285us (baseline) -> 98.7us
measured on TimelineSim; hardware rel err 9.9e-3 vs the fp32 reference.
"""
import sys

if "/opt/trn_rl_repo" not in sys.path:
    sys.path.insert(0, "/opt/trn_rl_repo")

import numpy as np
import ml_dtypes

N, C, H, W = 16, 128, 128, 128
CH = 32
NCORES = 8
SPC = N // NCORES          # samples per core
RS = W                     # stored row stride
SR = H + 4                 # stored rows: guard, zero, 128 data, zero, guard
GP = SR * RS               # elements per stored plane (16896)
SC = 16.0                  # fp8 scale for x and conv weights
BF = ml_dtypes.bfloat16
E4NP = ml_dtypes.float8_e4m3fn

# DoubleRow tap pairs: (tap_a, tap_b); taps are t = 3*dy + dx.
PAIRS = [(0, 3), (1, 4), (2, 5), (6, 8), (7, None)]


def _build(prelu1: float, prelu2: float):
    import concourse.mybir as mybir
    import concourse.tile as tile
    from concourse import bacc
    import bass_rust

    F32 = mybir.dt.float32
    F32R = mybir.dt.float32r
    BF16 = mybir.dt.bfloat16
    E4 = mybir.dt.float8e4
    AF = mybir.ActivationFunctionType
    ALU = mybir.AluOpType
    PM = mybir.MatmulPerfMode
    V = bass_rust.VecI64Pair

    nc = bacc.Bacc("TRN2", target_bir_lowering=False, debug=False,
                   num_devices=NCORES)

    xc_d = nc.dram_tensor("xc", [SPC, C, 2, GP], E4, kind="ExternalInput").ap()
    cw1_d = nc.dram_tensor("cw1", [C, 5, 2, C], E4, kind="ExternalInput").ap()
    w2m_d = nc.dram_tensor("w2m", [C, 5, 2, C], BF16, kind="ExternalInput").ap()
    w2s_d = nc.dram_tensor("w2s", [C, 9, C], F32, kind="ExternalInput").ap()
    i2_d = nc.dram_tensor("i2", [C, 2, C], E4, kind="ExternalInput").ap()
    ones1_d = nc.dram_tensor("ones1", [1, C], F32, kind="ExternalInput").ap()
    c1b_d = nc.dram_tensor("c1b", [C, 1], F32, kind="ExternalInput").ap()
    w1pT_d = nc.dram_tensor("w1pT", [SPC, C, CH], F32, kind="ExternalInput").ap()
    b1g_d = nc.dram_tensor("b1g", [SPC, CH, 1], F32, kind="ExternalInput").ap()
    w2aT_d = nc.dram_tensor("w2aT", [SPC, CH + 1, C], F32, kind="ExternalInput").ap()
    y_d = nc.dram_tensor("y", [SPC, C, H, W], BF16, kind="ExternalOutput").ap()

    with tile.TileContext(nc) as tc, (
        tc.tile_pool(name="wp", bufs=1)) as wp, (
        tc.tile_pool(name="xp", bufs=1)) as xp, (
        tc.tile_pool(name="hp", bufs=1)) as hpool, (
        tc.tile_pool(name="sp", bufs=1)) as spool, (
        tc.tile_pool(name="yp", bufs=4)) as ypool, (
        tc.tile_pool(name="pc", bufs=3, space="PSUM")) as pc, (
        tc.tile_pool(name="pv", bufs=1, space="PSUM")) as pv, (
        tc.tile_pool(name="pf", bufs=1, space="PSUM")) as pf:

        # --- static weights / constants ---
        cw1_t = wp.tile([C, 5, 2, C], E4, name="cw1t")
        w2m_t = wp.tile([C, 5, 2, C], BF16, name="w2mt")
        w2s_t = wp.tile([C, 9, C], F32, name="w2st")
        i2_t = wp.tile([C, 2, C], E4, name="i2t")
        ones1_t = wp.tile([1, C], F32, name="ones1t")
        c1b_t = wp.tile([C, 1], F32, name="c1bt")
        z9_t = wp.tile([C, 9], F32, name="z9t")
        strash = wp.tile([C, 32 * RS], E4, name="strash")
        wdum = wp.tile([C, 2, C], E4, name="wdum")
        nc.vector.memset(wdum[:], 0)
        nc.vector.memset(z9_t[:], 0)

        # per-sample tiles
        xc = [xp.tile([C, 2, GP], E4, name=f"xc{s}") for s in range(SPC)]
        h1 = [hpool.tile([C, GP], E4, name=f"h1_{s}") for s in range(SPC)]
        w2g = [wp.tile([C, 5, 2, C], E4, name=f"w2g{s}") for s in range(SPC)]
        w1pT_t = [wp.tile([C, CH], F32, name=f"w1pT{s}") for s in range(SPC)]
        b1g_t = [wp.tile([CH, 1], F32, name=f"b1g{s}") for s in range(SPC)]
        w2aT_t = [wp.tile([CH + 1, C], F32, name=f"w2aT{s}") for s in range(SPC)]
        spart = [spool.tile([C, 16], F32, name=f"spart{s}") for s in range(SPC)]
        sig = [spool.tile([C, 9], F32, name=f"sig{s}") for s in range(SPC)]
        red = [spool.tile([C, 4], F32, name=f"red{s}") for s in range(SPC)]
        x1sb = [spool.tile([C, 1], F32, name=f"x1sb{s}") for s in range(SPC)]
        a_aug = [spool.tile([CH + 1, 1], F32, name=f"aaug{s}") for s in range(SPC)]
        gprer = [spool.tile([1, C], F32, name=f"gprer{s}") for s in range(SPC)]
        gb = [spool.tile([C, C], F32, name=f"gb{s}") for s in range(SPC)]

        pdum = pv.tile([C, C], F32, name="pvs")
        for _ in range(44):
            nc.tensor.matmul(pdum[:, 0:C], wdum[:], wdum[:],
                             start=True, stop=True,
                             perf_mode=PM.DoubleRow)
        for s in range(SPC):
            # guard + zero rows of h1 (interior rewritten every sample)
            nc.vector.memset(h1[s][:, 0:2 * RS], 0)
            nc.vector.memset(h1[s][:, (SR - 2) * RS:], 0)
            nc.vector.memset(a_aug[s][CH:CH + 1, :], 1.0)

        # --- DMAs (SP queue) ---
        # plane-0 (conv input) bands first so PE starts asap; the C16
        # correction plane is only needed by the conv2 inject, so it
        # streams later.  Band k covers stored rows 33k .. 33k+36.
        def x_bands(s, ks, pl):
            for k in ks:
                a = RS * 33 * k
                b = min(GP, RS * (33 * k + 37))
                m = (a + b) // (2 * RS) * RS
                nc.sync.dma_start(xc[s][:, pl, a:m], xc_d[s, :, pl, a:m])
                nc.sync.dma_start(xc[s][:, pl, m:b], xc_d[s, :, pl, m:b])

        nc.scalar.dma_start(cw1_t[:], cw1_d)
        for (a, b) in ((0, 13), (13, 46), (46, 90), (90, SR)):
            nc.sync.dma_start(xc[0][:, 0, RS * a:RS * b],
                              xc_d[0, :, 0, RS * a:RS * b])
        nc.sync.dma_start(c1b_t[:], c1b_d)
        for (a, b) in ((0, 34), (34, 67), (67, 100), (100, SR)):
            nc.sync.dma_start(xc[1][:, 0, RS * a:RS * b],
                              xc_d[1, :, 0, RS * a:RS * b])
        nc.sync.dma_start(i2_t[:], i2_d)
        nc.sync.dma_start(w2m_t[:], w2m_d)
        nc.sync.dma_start(w2s_t[:], w2s_d)
        nc.sync.dma_start(ones1_t[:], ones1_d)
        for s in range(SPC):
            nc.sync.dma_start(w1pT_t[s][:], w1pT_d[s])
            nc.sync.dma_start(b1g_t[s][:], b1g_d[s])
            nc.sync.dma_start(w2aT_t[s][:], w2aT_d[s])
        for s in range(SPC):
            for (a, b) in ((0, GP // 2), (GP // 2, GP)):
                nc.sync.dma_start(xc[s][:, 1, a:b], xc_d[s, :, 1, a:b])

        def conv_group(s, g, base_ap, pstride, wt, inj_base=None,
                       split=False):
            """10 DR tap matmuls (+2 injects) + wrap fix-ups -> 2-bank psum.

            base_ap: AP anchored at the image plane start.  With split=True
            each psum bank gets its own edge fix so the epilogue can drain
            bank 0 while the PE still fills bank 1 (shorter tail)."""
            pp = pc.tile([C, 2, 4, W], F32, name="pp")
            b0 = base_ap.offset
            # ff layout [C, side, h, row]: per-side 8 rows are contiguous
            ff = pf.tile([C, 2, 2, 4], F32, name="ff")
            pstr = pp[:].ap[0][0]

            def half(h):
                c = 2 * g + h
                for p in range(5):
                    ta, tb = PAIRS[p]
                    dy0, dx0 = ta // 3, ta % 3
                    dstr = 0 if tb is None else (
                        (tb // 3 - dy0) * RS + (tb % 3 - dx0))
                    rhs = base_ap.copy()
                    rhs.ap = V([[pstride, C], [dstr, 2], [1, 4 * W]])
                    rhs.offset = b0 + (4 * c + dy0 + 1) * RS + dx0 - 1
                    nc.tensor.matmul(pp[:, h], wt[:, p], rhs,
                                     start=(p == 0), stop=False,
                                     perf_mode=PM.DoubleRow,
                                     skip_group_check=True)
                if inj_base is not None:   # conv2: inject residual x
                    inj = inj_base.copy()
                    inj.ap = V([[2 * GP, C], [GP, 2], [1, 4 * W]])
                    inj.offset = inj_base.offset + (4 * c + 2) * RS
                    nc.tensor.matmul(pp[:, h], i2_t[:], inj,
                                     start=False, stop=False,
                                     perf_mode=PM.DoubleRow,
                                     skip_group_check=True)

            def fix(h, nrows=4):
                # wrap-around garbage at out cols 0/127: accumulate garbage
                # into contiguous psum F, then subtract from the edge cols.
                # col 0: taps (dy,0) = pair0 + single t6 (= wt[:,3,0]);
                # col 127: taps (dy,2) = pair2 + single t8 (= wt[:,3,1]).
                # nrows=8 covers both psum banks with one matmul pair.
                for side, (pair_p, single_sl, coff) in enumerate(
                        ((0, (3, 0), -1), (2, (3, 1), RS))):
                    if nrows == 4:
                        fo = ff[:, side, h]
                    else:
                        fo = ff[:, side].rearrange("p a b -> p (a b)")
                    src = base_ap.copy()
                    src.ap = V([[pstride, C], [RS, 2], [RS, nrows]])
                    src.offset = b0 + (8 * g + 4 * h + 1) * RS + coff
                    nc.tensor.matmul(fo, wt[:, pair_p], src,
                                     start=True, stop=False,
                                     perf_mode=PM.DoubleRow,
                                     skip_group_check=True)
                    src2 = base_ap.copy()
                    src2.ap = V([[pstride, C], [RS, nrows]])
                    src2.offset = b0 + (8 * g + 4 * h + 3) * RS + coff
                    nc.tensor.matmul(fo, wt[:, single_sl[0], single_sl[1]],
                                     src2, start=False, stop=True,
                                     skip_group_check=True)

            def merge(hs):
                # DVE may read only ONE operand from PSUM: stage F in SBUF
                # (fs mirrors ff's flat [side, h, row] layout)
                nh = len(hs)
                fs = spool.tile([C, 16], F32, name="fs", bufs=4)
                fdst = fs[:, 0:1].copy()
                fdst.ap = V([[16, C], [8, 2], [1, 4 * nh]])
                fdst.offset = fdst.offset + 4 * hs[0]
                fsrc = ff[:, 0, 0, 0:1].copy()
                fsrc.ap = V([[16, C], [8, 2], [1, 4 * nh]])
                fsrc.offset = fsrc.offset + 4 * hs[0]
                nc.vector.tensor_scalar(fdst, fsrc, 0.0, None, ALU.add)
                edge = pp[:].copy()
                edge.ap = V([[pstr, C], [512, nh], [128, 4], [W - 1, 2]])
                edge.offset = pp[:].offset + 512 * hs[0]
                fap = fs[:, 0:1].copy()
                fap.ap = V([[16, C], [4, nh], [1, 4], [8, 2]])
                fap.offset = fap.offset + 4 * hs[0]
                nc.vector.tensor_tensor(edge, edge, fap, op=ALU.subtract)

            if split:
                half(0); fix(0); merge([0])
                half(1); fix(1); merge([1])
            else:
                half(0); half(1); fix(0, nrows=8); merge([0, 1])
            return pp

        def conv1_group(s, g):
            pp = conv_group(s, g, xc[s][:, 0, 0:1], 2 * GP, cw1_t)
            a0 = (8 * g + 2) * RS
            out = h1[s][:, a0:a0 + 8 * RS].rearrange(
                "p (a b w) -> p a b w", a=2, b=4)
            if g < 4:
                # early groups: pooling partial via DVE copy-to-trash with
                # accum (keeps the Act epilogue 187ns leaner; DVE has slack
                # early in the phase)
                nc.scalar.activation(out, pp[:], AF.Prelu, bias=c1b_t[:],
                                     scale=1.0 / (SC * SC), alpha=prelu1)
                nc.vector.tensor_scalar(strash[:, 0:8 * RS],
                                        h1[s][:, a0:a0 + 8 * RS],
                                        1.0, 0.0, ALU.mult, ALU.add,
                                        accum_out=spart[s][:, g:g + 1])
            else:
                nc.scalar.activation(out, pp[:], AF.Prelu, bias=c1b_t[:],
                                     scale=1.0 / (SC * SC), alpha=prelu1,
                                     accum_out=spart[s][:, g:g + 1])

        def conv2_group(s, g, on_dve, split=False, tail=False):
            pp = conv_group(s, g, h1[s][:, 0:1], GP, w2g[s],
                            inj_base=xc[s][:, 0, 0:1], split=split)
            if tail:
                # split=True ordered the halves' fixes/merges separately:
                # drain each bank through Act + sync store as soon as ready
                for h in range(2):
                    yh = ypool.tile([C, 4, W], BF16, name="yh")
                    nc.scalar.activation(yh[:], pp[:, h], AF.Prelu,
                                         scale=1.0 / SC, alpha=prelu2)
                    nc.sync.dma_start(y_d[s, :, 8 * g + 4 * h:
                                          8 * g + 4 * h + 4, :], yh[:])
                return
            if split:
                # per-half epilogue + store on disjoint engines/queues:
                # drains bank 0 while the PE still fills bank 1
                yh = ypool.tile([C, 4, W], BF16, name="yh")
                nc.scalar.activation(yh[:], pp[:, 0], AF.Prelu,
                                     scale=1.0 / SC, alpha=prelu2)
                nc.sync.dma_start(y_d[s, :, 8 * g:8 * g + 4, :], yh[:])
                th = ypool.tile([C, 4, W], BF16, name="th")
                nc.vector.tensor_scalar(th[:], pp[:, 1], 1.0 / SC, None,
                                        ALU.mult)
                yh2 = ypool.tile([C, 4, W], BF16, name="yh2")
                nc.vector.scalar_tensor_tensor(yh2[:], th[:], prelu2, th[:],
                                               op0=ALU.mult, op1=ALU.max)
                nc.gpsimd.dma_start(y_d[s, :, 8 * g + 4:8 * g + 8, :],
                                    yh2[:])
                return
            yt = ypool.tile([C, 2, 4, W], BF16, name="yt")
            if on_dve:
                tt = ypool.tile([C, 2, 4, W], BF16, name="tt")
                nc.vector.tensor_scalar(tt[:], pp[:], 1.0 / SC, None, ALU.mult)
                nc.vector.scalar_tensor_tensor(yt[:], tt[:], prelu2, tt[:],
                                               op0=ALU.mult, op1=ALU.max)
            else:
                nc.scalar.activation(yt[:], pp[:], AF.Prelu,
                                     scale=1.0 / SC, alpha=prelu2)
            nc.sync.dma_start(
                y_d[s, :, 8 * g:8 * g + 8, :].rearrange(
                    "p (a b) w -> p a b w", a=2), yt[:])

        def pool_ssum(s, q):
            # pooling sum partial over data rows 32q..32q+31 on idle GPSIMD:
            # copy-to-trash with accum_out gives the free-axis sum
            a = (2 + 32 * q) * RS
            nc.gpsimd.tensor_scalar(strash[:], h1[s][:, a:a + 32 * RS],
                                    1.0, None, ALU.mult,
                                    accum_out=spart[s][:, q:q + 1])

        def stats_sigma(s):
            """strips + corners + sigma build (DVE), inclusion-exclusion."""
            hs = h1[s]
            X = mybir.AxisListType.X
            nc.vector.tensor_reduce(red[s][:, 0:1], spart[s][:], axis=X,
                                    op=ALU.add)                           # S (from Pool partials)
            nc.vector.tensor_reduce(red[s][:, 1:2], hs[:, 2 * RS:3 * RS],
                                    axis=X, op=ALU.add)                   # Rt
            nc.vector.tensor_reduce(red[s][:, 2:3],
                                    hs[:, (SR - 3) * RS:(SR - 2) * RS],
                                    axis=X, op=ALU.add)                   # Rb
            cl = hs[:, 0:1].copy()
            cl.ap = V([[GP, C], [RS, H]])
            cl.offset = cl.offset + 2 * RS
            nc.vector.tensor_reduce(red[s][:, 3:4], cl, axis=X, op=ALU.add)  # Cl
            crt = spool.tile([C, 1], F32, name=f"cr{s}")
            cr = hs[:, 0:1].copy()
            cr.ap = V([[GP, C], [RS, H]])
            cr.offset = cr.offset + 2 * RS + (W - 1)
            nc.vector.tensor_reduce(crt[:], cr, axis=X, op=ALU.add)          # Cr
            sg = sig[s]
            nc.vector.tensor_scalar(sg[:], z9_t[:], red[s][:, 0:1], None,
                                    ALU.add)
            nc.vector.tensor_scalar(sg[:, 0:3], sg[:, 0:3], red[s][:, 2:3],
                                    None, ALU.subtract)                   # dy=0: -Rb
            nc.vector.tensor_scalar(sg[:, 6:9], sg[:, 6:9], red[s][:, 1:2],
                                    None, ALU.subtract)                   # dy=2: -Rt
            for col0, which in ((0, crt[:]), (2, red[s][:, 3:4])):
                ap = sg[:, col0:col0 + 1].copy()
                ap.ap = V([[9, C], [3, 3]])
                nc.vector.tensor_scalar(ap, ap, which, None, ALU.subtract)
            # corners [(0,0),(0,127),(127,0),(127,127)]: strided fp8->f32 copy
            corn = spool.tile([C, 2, 2], F32, name=f"corn{s}")
            cap = hs[:, 0:1].copy()
            cap.ap = V([[GP, C], [(H - 1) * RS, 2], [W - 1, 2]])
            cap.offset = cap.offset + 2 * RS
            nc.vector.tensor_scalar(corn[:], cap, 0.0, None, ALU.add)
            cf = corn[:].rearrange("p a b -> p (a b)")
            for t, ci in ((8, 0), (6, 1), (2, 2), (0, 3)):
                nc.vector.tensor_scalar(sg[:, t:t + 1], sg[:, t:t + 1],
                                        cf[:, ci:ci + 1], None, ALU.add)

        def gate_mlp(s):
            t1 = pv.tile([C, C], F32, name="pvs")
            psx, psa = t1[:, 0:1], t1[0:CH, 1:2]
            for t in range(9):
                nc.tensor.matmul(psx, w2s_t[:, t], sig[s][:, t:t + 1],
                                 start=(t == 0), stop=(t == 8))
            nc.vector.tensor_scalar(x1sb[s][:], psx, 0.0, None, ALU.add)
            nc.tensor.matmul(psa, w1pT_t[s][:], x1sb[s][:],
                             start=True, stop=True)
            nc.vector.tensor_scalar(a_aug[s][0:CH, :], psa, b1g_t[s][:],
                                    0.0, ALU.add, ALU.max)
            psg = pv.tile([C, C], F32, name="pvs")[0:1, :]
            nc.tensor.matmul(psg, a_aug[s][:], w2aT_t[s][:],
                             start=True, stop=True)
            nc.vector.tensor_scalar(gprer[s][:], psg, 0.0, None, ALU.add)
            psb = pv.tile([C, C], F32, name="pvs")
            nc.tensor.matmul(psb[:], ones1_t[:], gprer[s][:],
                             start=True, stop=True)
            nc.scalar.activation(gb[s][:], psb[:], AF.Sigmoid)

        def fold_w2(s):
            def bc(shape_dims):
                ap = gb[s][:, 0:1].copy()
                ap.ap = V([[C, C]] + shape_dims)
                return ap
            nc.gpsimd.tensor_tensor(w2g[s][:], w2m_t[:],
                                    bc([[0, 5], [0, 2], [1, C]]), op=ALU.mult)

        # ---------------- emission schedule ----------------
        for g in range(16):
            conv1_group(0, g)
        stats_sigma(0)
        for g in range(8):
            conv1_group(1, g)
        gate_mlp(0)
        for g in range(8, 12):
            conv1_group(1, g)
        fold_w2(0)
        for g in range(12, 16):
            conv1_group(1, g)
        stats_sigma(1)
        for g in range(8):
            conv2_group(0, g, on_dve=False)
        gate_mlp(1)
        for g in range(8, 12):
            conv2_group(0, g, on_dve=False)
        fold_w2(1)
        for g in range(12, 16):
            conv2_group(0, g, on_dve=False)
        for g in range(15):
            conv2_group(1, g, on_dve=False)
        conv2_group(1, 15, on_dve=False, split=True, tail=True)

    nc.compile()
    return nc


_CACHE = {}


def _get_program(prelu1, prelu2):
    key = (float(prelu1), float(prelu2))
    if key not in _CACHE:
        _CACHE[key] = _build(*key)
    return _CACHE[key]


def _prep(x, intensity, conv1_w, conv1_b, prelu1, conv2_w, conv2_b,
          aW1, ab1, aW2, ab2, prelu2):
    x = np.asarray(x, np.float32)
    idx = np.asarray(intensity).astype(np.int64) - 1
    conv1_w = np.asarray(conv1_w, np.float32)
    conv1_b = np.asarray(conv1_b, np.float32)
    conv2_w = np.asarray(conv2_w, np.float32)
    conv2_b = np.asarray(conv2_b, np.float32)
    aW1 = np.asarray(aW1, np.float32)
    ab1 = np.asarray(ab1, np.float32)
    aW2 = np.asarray(aW2, np.float32)
    ab2 = np.asarray(ab2, np.float32)
    assert not np.any(conv2_b), "conv2 bias folding not implemented"

    # stored planes: guard row, zero row, 128 data rows, zero row, guard row
    xpad = np.zeros((N, C, SR, RS), np.float32)
    xpad[:, :, 2:H + 2, :] = x * SC
    x16 = xpad.astype(E4NP)
    c16 = (xpad - x16.astype(np.float32)).astype(E4NP)
    xc = np.stack([x16, c16], axis=2).reshape(N, C, 2, GP)

    wtap1 = conv1_w.transpose(1, 2, 3, 0).reshape(C, 9, C)  # [i, t, o]
    wtap2 = conv2_w.transpose(1, 2, 3, 0).reshape(C, 9, C)

    def pair_pack(wtap, dtype):
        out = np.zeros((C, 5, 2, C), np.float32)
        for p, (ta, tb) in enumerate(PAIRS):
            out[:, p, 0] = wtap[:, ta]
            if tb is not None:
                out[:, p, 1] = wtap[:, tb]
        return np.ascontiguousarray((out * SC).astype(dtype))

    cw1 = pair_pack(wtap1, E4NP)
    w2m = pair_pack(wtap2, BF)
    w2s = np.ascontiguousarray(wtap2)
    i2 = np.zeros((C, 2, C), np.float32)
    i2[:, 0] = np.eye(C)
    i2[:, 1] = np.eye(C)
    i2 = i2.astype(E4NP)

    w1pT = np.ascontiguousarray(
        (aW1[idx] / (SC * H * W)).transpose(0, 2, 1))     # [N, C, CH]
    b1g = np.ascontiguousarray(ab1[idx])[:, :, None]      # [N, CH, 1]
    w2aT = np.concatenate(
        [aW2[idx].transpose(0, 2, 1), ab2[idx][:, None, :]], axis=1)

    nc = _get_program(float(prelu1), float(prelu2))

    in_maps = []
    for i in range(NCORES):
        sl = slice(i * SPC, (i + 1) * SPC)
        in_maps.append(dict(
            xc=xc[sl], cw1=cw1, w2m=w2m, w2s=w2s, i2=i2,
            ones1=np.ones((1, C), np.float32),
            c1b=conv1_b[:, None],
            w1pT=np.ascontiguousarray(w1pT[sl]),
            b1g=np.ascontiguousarray(b1g[sl]),
            w2aT=np.ascontiguousarray(w2aT[sl])))
    return nc, in_maps


def kernel(**inputs):
    import time
    from concourse.bass_utils import run_bass_kernel_spmd

    nc, in_maps = _prep(**inputs)
    res = None
    for attempt, pause in enumerate((0, 15, 60, 120)):
        if pause:
            time.sleep(pause)
        try:
            res = run_bass_kernel_spmd(nc, in_maps,
                                       core_ids=list(range(NCORES)))
            break
        except Exception:
            # transient NRT_EXEC_UNIT_UNRECOVERABLE (wedged core); retry
            if attempt == 3:
                raise
    return np.concatenate(
        [r["y"].astype(np.float32) for r in res.results], axis=0)
